# revision 6
# baseline (speedup 1.0000x reference)
"""Trainium2 Bass kernel for a 2-layer bidirectional LSTM encoder.

Problem: inputs [64, 512, 256] -> 2 stacked Bidirectional(LSTM(384)) layers
-> output [64, 512, 768] (Keras gate order i,f,g,o; sigmoid/tanh).

Strategy (8 NeuronCores, data-parallel over batch, 8 batch rows per core):
  * Everything on-chip is feature-major ("transposed"): features on the 128
    SBUF partitions, (time, batch) along the free dim.  This makes the gate
    elementwise work use all 128 vector/scalar lanes.
  * The input projections G = X @ Wk + b for all timesteps are precomputed
    with large weight-stationary matmuls and staged in DRAM (bf16).
  * The sequential recurrence then only does z_t = G_t + Wr^T h_{t-1} as 36
    small weight-stationary matmuls (12 output chunks x 3 contraction chunks)
    per direction per step, with fw/bw interleaved so the gate latency of one
    direction hides under the other direction's PE burst.
  * Host pre-permutes gates to [i, f, o, 2*g] so tanh(g) = 2*sigmoid(2g)-1
    turns ALL gate activations into a single Sigmoid instruction per step.
"""

import os
import sys

for _p in ("/opt/trn_rl_repo", "/root/.axon_site/_ro/trn_rl_repo"):
    if os.path.isdir(_p) and _p not in sys.path:
        sys.path.insert(0, _p)

import ml_dtypes
import numpy as np

import concourse.bass as bass
import concourse.mybir as mybir
import concourse.tile as tile
from concourse.bass_utils import run_bass_kernel_spmd


# ---------------------------------------------------------------------------
# Workaround: walrus CoreV3 rejects the Tile tail Drain when it carries more
# than one sem wait ("Too many sync wait commands").  Redistribute the waits
# onto single-wait SP nops.
# ---------------------------------------------------------------------------
def _apply_tile_drain_fix():
    from concourse.vector_clock import ScopedClock

    if getattr(tile.TileContext, "_drain_fix_applied", False):
        return

    def _drain_and_barrier(self, tick_clock, wait_clock):
        nc = self.nc
        drain_inst = nc.sync.drain()
        wait_clock.add_sem_waits(
            drain_inst.ins, ScopedClock({None: tick_clock.global_clock})
        )
        si = drain_inst.ins.sync_info
        if si is not None and si.on_wait:
            waits = list(si.on_wait)
            ups = list(si.on_update) if si.on_update else []
            drain_inst.ins.sync_info = mybir.SyncInfo(on_wait=[], on_update=ups)
            for w in waits:
                n = nc.sync.nop()
                n.ins.sync_info = mybir.SyncInfo(on_wait=[w], on_update=[])

        nc.all_engine_barrier()
        assert self.sems is not None
        popped = nc._tile_sem_poison_stack.pop()
        assert popped is self._sem_poison
        nc.clear_and_free_semaphores(list(self.sems.allocated().values()))
        nc.all_engine_barrier()

    tile.TileContext._drain_and_barrier = _drain_and_barrier
    tile.TileContext._drain_fix_applied = True


_apply_tile_drain_fix()


def _split_excess_waits(nc, maxw=1):
    """walrus CoreV2/V3 codegen rejects instructions carrying more than one
    sem wait ("Too many sync wait commands").  Move excess waits onto NoOps
    inserted immediately before the instruction on the same engine."""
    k = 0
    for fn in nc.m.functions:
        for bb in fn.blocks:
            insts = list(bb.instructions)
            out = []
            changed = False
            for inst in insts:
                si = getattr(inst, "sync_info", None)
                if si is not None and si.on_wait and len(si.on_wait) > maxw:
                    waits = list(si.on_wait)
                    ups = list(si.on_update) if si.on_update else []
                    for w in waits[maxw:]:
                        n = mybir.InstNoOp(name=f"xwait_{k}")
                        k += 1
                        n.engine = inst.engine
                        n.sync_info = mybir.SyncInfo(on_wait=[w], on_update=[])
                        out.append(n)
                    inst.sync_info = mybir.SyncInfo(on_wait=waits[:maxw],
                                                    on_update=ups)
                    changed = True
                out.append(inst)
            if changed:
                bb.instructions = out


# ---------------------------------------------------------------------------
# Problem constants
# ---------------------------------------------------------------------------
B, T_FULL, D, H = 64, 512, 256, 384
NCORES = 8
BL = B // NCORES          # 8 batch rows per core
NH = H // 128             # 3 recurrent contraction chunks
NM = 4 * H // 128         # 12 output (gate-feature) chunks
F32 = mybir.dt.float32
BF16 = mybir.dt.bfloat16
AF = mybir.ActivationFunctionType
ALU = mybir.AluOpType
BF16_NP = ml_dtypes.bfloat16


def build_program(T=T_FULL, TB=32):
    """Build the single-core Bass/Tile program (same NEFF runs SPMD on 8 cores)."""
    assert T % TB == 0
    NCH = (T * BL) // 512        # 512-wide column chunks of the (t, b) axis
    NKS = {0: D // 128, 1: 2 * H // 128}   # Wk contraction chunks per layer

    nc = bass.Bass("TRN2", target_bir_lowering=False, debug=False)

    # ---------------- DRAM I/O ----------------
    xT = nc.dram_tensor("xT", [D // 128, 128, T * BL], BF16, kind="ExternalInput")
    # feature-major output: out[d, j, p, t*BL + b]; host transposes to [BL, T, 2H]
    out_d = nc.dram_tensor("out", [2, NH, 128, T * BL], F32, kind="ExternalOutput")

    wk_d, wr_d, bias_d = {}, {}, {}
    for l in range(2):
        for d in range(2):
            nk = NKS[l]
            wk_d[l, d] = nc.dram_tensor(f"wk{l}{d}", [nk, 128, 4 * H], BF16,
                                        kind="ExternalInput")
            wr_d[l, d] = nc.dram_tensor(f"wr{l}{d}", [NH, 128, 4 * H], BF16,
                                        kind="ExternalInput")
            bias_d[l, d] = nc.dram_tensor(f"bias{l}{d}", [128, NM], F32,
                                          kind="ExternalInput")

    with tile.TileContext(nc) as tc, \
         tc.tile_pool(name="persist", bufs=1) as persist, \
         tc.tile_pool(name="wkp", bufs=2) as wkp, \
         tc.tile_pool(name="wrp", bufs=2) as wrp, \
         tc.tile_pool(name="gblk", bufs=2) as gblk, \
         tc.tile_pool(name="gstage", bufs=2) as gstage, \
         tc.tile_pool(name="step", bufs=3) as stepp, \
         tc.tile_pool(name="small", bufs=4) as small, \
         tc.tile_pool(name="cells", bufs=2) as cells, \
         tc.tile_pool(name="oblk", bufs=2) as oblk, \
         tc.tile_pool(name="zpsum", bufs=4, space="PSUM") as zpsum, \
         tc.tile_pool(name="ppsum", bufs=4, space="PSUM") as ppsum, \
         tc.tile_pool(name="gdram", bufs=1, space="DRAM") as gdram:

        # ---------------- constants / persistent tiles ----------------
        zero_h = persist.tile([128, BL], BF16, tag="zeroh")
        nc.vector.memset(zero_h, 0.0)

        bias_sb = {}
        for l in range(2):
            for d in range(2):
                bias_sb[l, d] = persist.tile([128, NM], F32, tag=f"bias{l}{d}", name=f"bias_sb{l}{d}")
                nc.sync.dma_start(out=bias_sb[l, d][:], in_=bias_d[l, d][:, :])

        # layer-0 input, feature-major, bf16 (host pre-transposed)
        x0t = persist.tile([128, D // 128, T * BL], BF16, tag="x0t")
        for k in range(D // 128):
            nc.sync.dma_start(out=x0t[:, k, :], in_=xT[k, :, :])

        # ---------------- helpers ----------------
        def load_wk(l):
            tiles = {}
            for d in range(2):
                nk = NKS[l]
                w = wkp.tile([128, NKS[1], 4 * H], BF16, tag="wk", name=f"wk_sb{l}{d}")
                for k in range(nk):
                    nc.sync.dma_start(out=w[:, k, :], in_=wk_d[l, d][k, :, :])
                tiles[d] = w
            return tiles

        def load_wr(l):
            tiles = {}
            for d in range(2):
                w = wrp.tile([128, NH, 4 * H], BF16, tag="wr", name=f"wr_sb{l}{d}")
                for k in range(NH):
                    nc.sync.dma_start(out=w[:, k, :], in_=wr_d[l, d][k, :, :])
                tiles[d] = w
            return tiles

        def precompute_G(l, wk_sb, rhs_fn):
            """G[d] = (X @ Wk'[d] + b'[d])^T staged to DRAM as [NM, 128, T*BL] bf16.

            rhs_fn(d, k, n) -> AP [128, 512] bf16: columns n*512..(n+1)*512 of
            the feature-major layer input, contraction chunk k.
            """
            nk = NKS[l]
            gd = {}
            for d in range(2):
                gd[d] = gdram.tile([NM, 128, T * BL], BF16, tag=f"g{l}{d}", name=f"gdram{l}{d}")
                for m in range(NM):
                    stage = gstage.tile([128, min(NCH, 4) * 512], BF16, tag="gs")
                    for ng in range((NCH + 3) // 4):
                        nlo = ng * 4
                        nhi = min(nlo + 4, NCH)
                        pss = []
                        for n in range(nlo, nhi):
                            ps = ppsum.tile([128, 512], F32, tag="pp")
                            pss.append(ps)
                            for k in range(nk):
                                nc.tensor.matmul(
                                    ps[:],
                                    wk_sb[d][:, k, m * 128:(m + 1) * 128],
                                    rhs_fn(d, k, n),
                                    start=(k == 0), stop=(k == nk - 1),
                                )
                        if ng > 0:
                            stage = gstage.tile([128, min(NCH, 4) * 512], BF16,
                                                tag="gs")
                        for i, n in enumerate(range(nlo, nhi)):
                            nc.vector.tensor_scalar_add(
                                out=stage[:, i * 512:(i + 1) * 512],
                                in0=pss[i][:],
                                scalar1=bias_sb[l, d][:, m:m + 1],
                            )
                        nc.sync.dma_start(
                            out=gd[d][m, :, nlo * 512:nhi * 512],
                            in_=stage[:, :(nhi - nlo) * 512],
                        )
            return gd

        def recurrence(l, wr_sb, g_d, hout, is_last):
            """Run T bidirectional LSTM steps for layer l.

            hout: [128, 2, NH, T, BL] bf16 tile; h_t written feature-major.
            If is_last, also write fp32 h to the output scatter blocks.
            """
            cprev = cells.tile([128, 2, NH, BL], F32, tag="c")
            nc.vector.memset(cprev, 0.0)

            for blk in range(T // TB):
                gf = gblk.tile([128, NM, TB * BL], BF16, tag="gf")
                gb = gblk.tile([128, NM, TB * BL], BF16, tag="gb")
                c0 = blk * TB * BL
                nc.sync.dma_start(
                    out=gf[:],
                    in_=g_d[0][:, :, c0:c0 + TB * BL].rearrange("c p n -> p c n"))
                rb0 = T * BL - c0 - TB * BL
                nc.sync.dma_start(
                    out=gb[:],
                    in_=g_d[1][:, :, rb0:rb0 + TB * BL].rearrange("c p n -> p c n"))

                if is_last:
                    of = oblk.tile([128, TB, NH, BL], F32, tag="of")
                    ob = oblk.tile([128, TB, NH, BL], F32, tag="ob")

                for s_ in range(TB):
                    s = blk * TB + s_
                    tf = s              # fw actual time index
                    tbw = T - 1 - s     # bw actual time index

                    zp = zpsum.tile([128, 2, NM, BL], F32, tag="zp")
                    for d in range(2):
                        tprev = tf - 1 if d == 0 else tbw + 1
                        for c in range(NM):
                            for k in range(NH):
                                rhs = (zero_h[:, :] if s == 0
                                       else hout[:, d, k, tprev, :])
                                nc.tensor.matmul(
                                    zp[:, d, c, :],
                                    wr_sb[d][:, k, c * 128:(c + 1) * 128],
                                    rhs,
                                    start=(k == 0), stop=(k == NH - 1),
                                )

                    a0 = stepp.tile([128, 2, NM, BL], F32, tag="a0")
                    nc.vector.tensor_tensor(
                        a0[:, 0], zp[:, 0],
                        gf[:, :, s_ * BL:(s_ + 1) * BL], ALU.add)
                    nc.vector.tensor_tensor(
                        a0[:, 1], zp[:, 1],
                        gb[:, :, (TB - 1 - s_) * BL:(TB - s_) * BL], ALU.add)

                    a1 = stepp.tile([128, 2, NM, BL], F32, tag="a1")
                    nc.scalar.activation(a1[:], a0[:], AF.Sigmoid)

                    # g' = tanh(g) = 2*sigmoid(2g) - 1  (2x folded into weights)
                    gp = small.tile([128, 2, NH, BL], F32, tag="gp")
                    nc.vector.tensor_scalar(
                        out=gp[:], in0=a1[:, :, 9:12, :],
                        scalar1=2.0, scalar2=1.0, op0=ALU.mult, op1=ALU.subtract)

                    t1 = small.tile([128, 2, NH, BL], F32, tag="t1")
                    nc.vector.tensor_tensor(t1[:], a1[:, :, 0:3, :], gp[:], ALU.mult)
                    t2 = small.tile([128, 2, NH, BL], F32, tag="t2")
                    nc.vector.tensor_tensor(t2[:], a1[:, :, 3:6, :], cprev[:], ALU.mult)
                    cn = cells.tile([128, 2, NH, BL], F32, tag="c")
                    nc.vector.tensor_tensor(cn[:], t1[:], t2[:], ALU.add)

                    th = small.tile([128, 2, NH, BL], F32, tag="th")
                    nc.scalar.activation(th[:], cn[:], AF.Tanh)

                    if is_last:
                        # h = o * tanh(c), fp32 straight into the output blocks
                        nc.vector.tensor_tensor(
                            of[:, s_], a1[:, 0, 6:9, :], th[:, 0], ALU.mult)
                        nc.vector.tensor_tensor(
                            ob[:, TB - 1 - s_], a1[:, 1, 6:9, :], th[:, 1], ALU.mult)
                        nc.vector.tensor_copy(hout[:, 0, :, tf, :], of[:, s_])
                        nc.vector.tensor_copy(hout[:, 1, :, tbw, :],
                                              ob[:, TB - 1 - s_])
                    else:
                        h32 = small.tile([128, 2, NH, BL], F32, tag="h32")
                        nc.vector.tensor_tensor(h32[:], a1[:, :, 6:9, :], th[:],
                                                ALU.mult)
                        nc.vector.tensor_copy(hout[:, 0, :, tf, :], h32[:, 0])
                        nc.vector.tensor_copy(hout[:, 1, :, tbw, :], h32[:, 1])

                    cprev = cn

                if is_last:
                    c0f = blk * TB * BL
                    c0b = T * BL - c0f - TB * BL
                    for j in range(NH):
                        nc.sync.dma_start(
                            out=out_d[0, j, :, c0f:c0f + TB * BL],
                            in_=of[:, :, j, :])
                        nc.sync.dma_start(
                            out=out_d[1, j, :, c0b:c0b + TB * BL],
                            in_=ob[:, :, j, :])

        # ---------------- phases ----------------
        with nc.named_scope("G0"):
            wk0 = load_wk(0)
            g0 = precompute_G(
                0, wk0,
                lambda d, k, n: x0t[:, k, n * 512:(n + 1) * 512])

        with nc.named_scope("L0"):
            wr0 = load_wr(0)
            x1t = persist.tile([128, 2, NH, T, BL], BF16, tag="hfull")
            recurrence(0, wr0, g0, x1t, is_last=False)

        with nc.named_scope("G1"):
            wk1 = load_wk(1)

            def rhs1(d, k, n):
                kk = k  # contraction index over 2H = (dir, j)
                dd, jj = kk // NH, kk % NH
                flat = x1t[:, dd, jj, :, :].rearrange("p t b -> p (t b)")
                return flat[:, n * 512:(n + 1) * 512]

            g1 = precompute_G(1, wk1, rhs1)

        with nc.named_scope("L1"):
            wr1 = load_wr(1)
            h1 = persist.tile([128, 2, NH, T, BL], BF16, tag="hfull")
            recurrence(1, wr1, g1, h1, is_last=True)

    _split_excess_waits(nc)
    return nc


# ---------------------------------------------------------------------------
# Host-side input preparation
# ---------------------------------------------------------------------------
def _prep_weights(Wk, Wr, b):
    """Permute gate blocks [i,f,g,o] -> [i,f,o,2g]; return device arrays."""
    def perm(w):
        i, f, g, o = (w[..., 0:H], w[..., H:2 * H],
                      w[..., 2 * H:3 * H], w[..., 3 * H:4 * H])
        return np.concatenate([i, f, o, 2.0 * g], axis=-1)

    Wkp = perm(np.asarray(Wk, np.float32))
    Wrp = perm(np.asarray(Wr, np.float32))
    bp = perm(np.asarray(b, np.float32))
    nk = Wkp.shape[0] // 128
    wk_dev = np.ascontiguousarray(Wkp.reshape(nk, 128, 4 * H)).astype(BF16_NP)
    wr_dev = np.ascontiguousarray(Wrp.reshape(NH, 128, 4 * H)).astype(BF16_NP)
    bias_dev = np.ascontiguousarray(bp.reshape(NM, 128).T).astype(np.float32)
    return wk_dev, wr_dev, bias_dev


def make_in_maps(inputs, T=T_FULL):
    x = np.asarray(inputs["inputs"], np.float32)   # [B, T, D]
    weights = {}
    for l in range(2):
        for di, dn in enumerate(("fw", "bw")):
            wk, wr, bias = _prep_weights(inputs[f"Wk{l}_{dn}"],
                                         inputs[f"Wr{l}_{dn}"],
                                         inputs[f"b{l}_{dn}"])
            weights[f"wk{l}{di}"] = wk
            weights[f"wr{l}{di}"] = wr
            weights[f"bias{l}{di}"] = bias

    in_maps = []
    for c in range(NCORES):
        xc = x[c * BL:(c + 1) * BL]                        # [BL, T, D]
        xt = np.ascontiguousarray(xc.transpose(2, 1, 0))   # [D, T, BL]
        xt = xt.reshape(D // 128, 128, T * BL).astype(BF16_NP)
        m = {"xT": xt}
        m.update(weights)
        in_maps.append(m)
    return in_maps


_PROGRAM_CACHE = {}


def _get_program(T=T_FULL):
    if T not in _PROGRAM_CACHE:
        _PROGRAM_CACHE[T] = build_program(T=T)
    return _PROGRAM_CACHE[T]


def run(inputs, T=T_FULL, **kw):
    nc = _get_program(T)
    in_maps = make_in_maps(inputs, T=T)
    res = run_bass_kernel_spmd(nc, in_maps, core_ids=list(range(NCORES)), **kw)
    outs = []
    for r in res.results:
        o = r["out"].reshape(2, NH, 128, T, BL)       # [d, j, p, t, b]
        o = o.transpose(4, 3, 0, 1, 2)                # [b, t, d, j, p]
        outs.append(np.ascontiguousarray(o.reshape(BL, T, 2 * H)))
    out = np.concatenate(outs, axis=0)
    return out, res


def kernel(**inputs):
    out, _ = run(inputs)
    return out


if __name__ == "__main__":
    import time

    t0 = time.time()
    nc = _get_program()
    print(f"build took {time.time() - t0:.1f}s")


# revision 8
# speedup vs baseline: 1.1048x; 1.1048x over previous
"""Trainium2 Bass kernel for a 2-layer bidirectional LSTM encoder.

Problem: inputs [64, 512, 256] -> 2 stacked Bidirectional(LSTM(384)) layers
-> output [64, 512, 768] (Keras gate order i,f,g,o; sigmoid/tanh).

Strategy (8 NeuronCores, data-parallel over batch, 8 batch rows per core):
  * Everything on-chip is feature-major ("transposed"): features on the 128
    SBUF partitions, (time, batch) along the free dim.  This makes the gate
    elementwise work use all 128 vector/scalar lanes.
  * The input projections G = X @ Wk + b for all timesteps are precomputed
    with large weight-stationary matmuls and staged in DRAM (bf16).
  * The sequential recurrence then only does z_t = G_t + Wr^T h_{t-1} as 36
    small weight-stationary matmuls (12 output chunks x 3 contraction chunks)
    per direction per step, with fw/bw interleaved so the gate latency of one
    direction hides under the other direction's PE burst.
  * Host pre-permutes gates to [i, f, o, 2*g] so tanh(g) = 2*sigmoid(2g)-1
    turns ALL gate activations into a single Sigmoid instruction per step.
"""

import os
import sys

for _p in ("/opt/trn_rl_repo", "/root/.axon_site/_ro/trn_rl_repo"):
    if os.path.isdir(_p) and _p not in sys.path:
        sys.path.insert(0, _p)

import ml_dtypes
import numpy as np

import concourse.bass as bass
import concourse.mybir as mybir
import concourse.tile as tile
from concourse.bass_utils import run_bass_kernel_spmd


# ---------------------------------------------------------------------------
# Workaround: walrus CoreV3 rejects the Tile tail Drain when it carries more
# than one sem wait ("Too many sync wait commands").  Redistribute the waits
# onto single-wait SP nops.
# ---------------------------------------------------------------------------
def _apply_tile_drain_fix():
    from concourse.vector_clock import ScopedClock

    if getattr(tile.TileContext, "_drain_fix_applied", False):
        return

    def _drain_and_barrier(self, tick_clock, wait_clock):
        nc = self.nc
        drain_inst = nc.sync.drain()
        wait_clock.add_sem_waits(
            drain_inst.ins, ScopedClock({None: tick_clock.global_clock})
        )
        si = drain_inst.ins.sync_info
        if si is not None and si.on_wait:
            waits = list(si.on_wait)
            ups = list(si.on_update) if si.on_update else []
            drain_inst.ins.sync_info = mybir.SyncInfo(on_wait=[], on_update=ups)
            for w in waits:
                n = nc.sync.nop()
                n.ins.sync_info = mybir.SyncInfo(on_wait=[w], on_update=[])

        nc.all_engine_barrier()
        assert self.sems is not None
        popped = nc._tile_sem_poison_stack.pop()
        assert popped is self._sem_poison
        nc.clear_and_free_semaphores(list(self.sems.allocated().values()))
        nc.all_engine_barrier()

    tile.TileContext._drain_and_barrier = _drain_and_barrier
    tile.TileContext._drain_fix_applied = True


_apply_tile_drain_fix()


def _split_excess_waits(nc, maxw=1):
    """walrus CoreV2/V3 codegen rejects instructions carrying more than one
    sem wait ("Too many sync wait commands").  Move excess waits onto NoOps
    inserted immediately before the instruction on the same engine."""
    k = 0
    for fn in nc.m.functions:
        for bb in fn.blocks:
            insts = list(bb.instructions)
            out = []
            changed = False
            for inst in insts:
                si = getattr(inst, "sync_info", None)
                if si is not None and si.on_wait and len(si.on_wait) > maxw:
                    waits = list(si.on_wait)
                    ups = list(si.on_update) if si.on_update else []
                    for w in waits[maxw:]:
                        n = mybir.InstNoOp(name=f"xwait_{k}")
                        k += 1
                        n.engine = inst.engine
                        n.sync_info = mybir.SyncInfo(on_wait=[w], on_update=[])
                        out.append(n)
                    inst.sync_info = mybir.SyncInfo(on_wait=waits[:maxw],
                                                    on_update=ups)
                    changed = True
                out.append(inst)
            if changed:
                bb.instructions = out


# ---------------------------------------------------------------------------
# Problem constants
# ---------------------------------------------------------------------------
B, T_FULL, D, H = 64, 512, 256, 384
NCORES = 8
BL = B // NCORES          # 8 batch rows per core
NH = H // 128             # 3 recurrent contraction chunks
NM = 4 * H // 128         # 12 output (gate-feature) chunks
F32 = mybir.dt.float32
BF16 = mybir.dt.bfloat16
AF = mybir.ActivationFunctionType
ALU = mybir.AluOpType
BF16_NP = ml_dtypes.bfloat16


def build_program(T=T_FULL, TB=32):
    """Build the single-core Bass/Tile program (same NEFF runs SPMD on 8 cores)."""
    assert T % TB == 0
    NCH = (T * BL) // 512        # 512-wide column chunks of the (t, b) axis
    NKS = {0: D // 128, 1: 2 * H // 128}   # Wk contraction chunks per layer

    nc = bass.Bass("TRN2", target_bir_lowering=False, debug=False)

    # ---------------- DRAM I/O ----------------
    xT = nc.dram_tensor("xT", [D // 128, 128, T * BL], BF16, kind="ExternalInput")
    # feature-major output: out[d, j, p, t*BL + b]; host transposes to [BL, T, 2H]
    out_d = nc.dram_tensor("out", [2, NH, 128, T * BL], F32, kind="ExternalOutput")

    wk_d, wr_d, bias_d = {}, {}, {}
    for l in range(2):
        for d in range(2):
            nk = NKS[l]
            wk_d[l, d] = nc.dram_tensor(f"wk{l}{d}", [nk, 128, 4 * H], BF16,
                                        kind="ExternalInput")
            wr_d[l, d] = nc.dram_tensor(f"wr{l}{d}", [NH, 128, 4 * H], BF16,
                                        kind="ExternalInput")
            bias_d[l, d] = nc.dram_tensor(f"bias{l}{d}", [128, NM], F32,
                                          kind="ExternalInput")

    with tile.TileContext(nc) as tc, \
         tc.tile_pool(name="persist", bufs=1) as persist, \
         tc.tile_pool(name="wkp", bufs=2) as wkp, \
         tc.tile_pool(name="wrp", bufs=2) as wrp, \
         tc.tile_pool(name="gblk", bufs=2) as gblk, \
         tc.tile_pool(name="gstage", bufs=2) as gstage, \
         tc.tile_pool(name="step", bufs=3) as stepp, \
         tc.tile_pool(name="small", bufs=4) as small, \
         tc.tile_pool(name="cells", bufs=2) as cells, \
         tc.tile_pool(name="oblk", bufs=2) as oblk, \
         tc.tile_pool(name="zpsum", bufs=2, space="PSUM") as zpsum, \
         tc.tile_pool(name="ppsum", bufs=4, space="PSUM") as ppsum, \
         tc.tile_pool(name="gdram", bufs=1, space="DRAM") as gdram:

        # ---------------- constants / persistent tiles ----------------
        zero_h = persist.tile([128, BL], BF16, tag="zeroh")
        nc.vector.memset(zero_h, 0.0)

        bias_sb = {}
        for l in range(2):
            for d in range(2):
                bias_sb[l, d] = persist.tile([128, NM], F32, tag=f"bias{l}{d}", name=f"bias_sb{l}{d}")
                nc.sync.dma_start(out=bias_sb[l, d][:], in_=bias_d[l, d][:, :])

        # layer-0 input, feature-major, bf16 (host pre-transposed)
        x0t = persist.tile([128, D // 128, T * BL], BF16, tag="x0t")
        for k in range(D // 128):
            nc.sync.dma_start(out=x0t[:, k, :], in_=xT[k, :, :])

        # ---------------- helpers ----------------
        def load_wk(l):
            tiles = {}
            for d in range(2):
                nk = NKS[l]
                w = wkp.tile([128, NKS[1], 4 * H], BF16, tag="wk", name=f"wk_sb{l}{d}")
                for k in range(nk):
                    nc.sync.dma_start(out=w[:, k, :], in_=wk_d[l, d][k, :, :])
                tiles[d] = w
            return tiles

        def load_wr(l):
            tiles = {}
            for d in range(2):
                w = wrp.tile([128, NH, 4 * H], BF16, tag="wr", name=f"wr_sb{l}{d}")
                for k in range(NH):
                    nc.sync.dma_start(out=w[:, k, :], in_=wr_d[l, d][k, :, :])
                tiles[d] = w
            return tiles

        def precompute_G(l, wk_sb, rhs_fn):
            """G[d] = (X @ Wk'[d] + b'[d])^T staged to DRAM as [NM, 128, T*BL] bf16.

            rhs_fn(d, k, n) -> AP [128, 512] bf16: columns n*512..(n+1)*512 of
            the feature-major layer input, contraction chunk k.
            """
            nk = NKS[l]
            gd = {}
            for d in range(2):
                gd[d] = gdram.tile([NM, 128, T * BL], BF16, tag=f"g{l}{d}", name=f"gdram{l}{d}")
                for m in range(NM):
                    stage = gstage.tile([128, min(NCH, 4) * 512], BF16, tag="gs")
                    for ng in range((NCH + 3) // 4):
                        nlo = ng * 4
                        nhi = min(nlo + 4, NCH)
                        pss = []
                        for n in range(nlo, nhi):
                            ps = ppsum.tile([128, 512], F32, tag="pp")
                            pss.append(ps)
                            for k in range(nk):
                                nc.tensor.matmul(
                                    ps[:],
                                    wk_sb[d][:, k, m * 128:(m + 1) * 128],
                                    rhs_fn(d, k, n),
                                    start=(k == 0), stop=(k == nk - 1),
                                )
                        if ng > 0:
                            stage = gstage.tile([128, min(NCH, 4) * 512], BF16,
                                                tag="gs")
                        for i, n in enumerate(range(nlo, nhi)):
                            nc.vector.tensor_scalar_add(
                                out=stage[:, i * 512:(i + 1) * 512],
                                in0=pss[i][:],
                                scalar1=bias_sb[l, d][:, m:m + 1],
                            )
                        nc.sync.dma_start(
                            out=gd[d][m, :, nlo * 512:nhi * 512],
                            in_=stage[:, :(nhi - nlo) * 512],
                        )
            return gd

        def recurrence(l, wr_sb, g_d, hout, is_last):
            """Run T bidirectional LSTM steps for layer l.

            hout: [128, 2, NH, T, BL] bf16 tile; h_t written feature-major.
            If is_last, also write fp32 h to the output scatter blocks.
            """
            cprev = {}
            for d in range(2):
                cprev[d] = cells.tile([128, NH, BL], F32, tag=f"c{d}",
                                      name=f"cinit{d}")
                nc.vector.memset(cprev[d], 0.0)

            for blk in range(T // TB):
                gf = gblk.tile([128, NM, TB * BL], BF16, tag="gf")
                gb = gblk.tile([128, NM, TB * BL], BF16, tag="gb")
                c0 = blk * TB * BL
                nc.sync.dma_start(
                    out=gf[:],
                    in_=g_d[0][:, :, c0:c0 + TB * BL].rearrange("c p n -> p c n"))
                rb0 = T * BL - c0 - TB * BL
                nc.sync.dma_start(
                    out=gb[:],
                    in_=g_d[1][:, :, rb0:rb0 + TB * BL].rearrange("c p n -> p c n"))

                if is_last:
                    of = oblk.tile([128, TB, NH, BL], F32, tag="of")
                    ob = oblk.tile([128, TB, NH, BL], F32, tag="ob")

                for s_ in range(TB):
                    s = blk * TB + s_
                    # fw processes t=s; bw processes t=T-1-s.  The two
                    # directions are emitted as separate instruction chains so
                    # one direction's PE burst overlaps the other's gate-chain
                    # latency (DVE/ACT).
                    for d in range(2):
                        t_d = s if d == 0 else T - 1 - s
                        tprev = t_d - 1 if d == 0 else t_d + 1
                        gsl = (gf[:, :, s_ * BL:(s_ + 1) * BL] if d == 0 else
                               gb[:, :, (TB - 1 - s_) * BL:(TB - s_) * BL])

                        zp = zpsum.tile([128, NM, BL], F32, tag=f"zp{d}",
                                        name=f"zp{d}_{s}")
                        for c in range(NM):
                            for k in range(NH):
                                rhs = (zero_h[:, :] if s == 0
                                       else hout[:, d, k, tprev, :])
                                nc.tensor.matmul(
                                    zp[:, c, :],
                                    wr_sb[d][:, k, c * 128:(c + 1) * 128],
                                    rhs,
                                    start=(k == 0), stop=(k == NH - 1),
                                )

                        a1 = stepp.tile([128, NM, BL], F32, tag=f"a1{d}",
                                        name=f"a1{d}_{s}")
                        nc.vector.tensor_tensor(a1[:], zp[:], gsl, ALU.add)
                        nc.scalar.activation(a1[:], a1[:], AF.Sigmoid)

                        # g' = tanh(g) = 2*sigmoid(2g) - 1 (2x folded into W)
                        gp = small.tile([128, NH, BL], F32, tag=f"gp{d}",
                                        name=f"gp{d}_{s}")
                        nc.vector.tensor_scalar(
                            out=gp[:], in0=a1[:, 9:12, :],
                            scalar1=2.0, scalar2=1.0,
                            op0=ALU.mult, op1=ALU.subtract)

                        t1 = small.tile([128, NH, BL], F32, tag=f"t1{d}",
                                        name=f"t1{d}_{s}")
                        nc.vector.tensor_tensor(t1[:], a1[:, 0:3, :], gp[:],
                                                ALU.mult)
                        t2 = small.tile([128, NH, BL], F32, tag=f"t2{d}",
                                        name=f"t2{d}_{s}")
                        nc.vector.tensor_tensor(t2[:], a1[:, 3:6, :],
                                                cprev[d][:], ALU.mult)
                        cn = cells.tile([128, NH, BL], F32, tag=f"c{d}",
                                        name=f"c{d}_{s}")
                        nc.vector.tensor_tensor(cn[:], t1[:], t2[:], ALU.add)

                        th = small.tile([128, NH, BL], F32, tag=f"th{d}",
                                        name=f"th{d}_{s}")
                        nc.scalar.activation(th[:], cn[:], AF.Tanh)

                        # critical path first: bf16 h feeds the next matmul
                        nc.vector.tensor_tensor(hout[:, d, :, t_d, :],
                                                a1[:, 6:9, :], th[:], ALU.mult)
                        if is_last:
                            osl = of[:, s_] if d == 0 else ob[:, TB - 1 - s_]
                            nc.vector.tensor_tensor(osl, a1[:, 6:9, :], th[:],
                                                    ALU.mult)
                        cprev[d] = cn

                if is_last:
                    c0f = blk * TB * BL
                    c0b = T * BL - c0f - TB * BL
                    for j in range(NH):
                        nc.sync.dma_start(
                            out=out_d[0, j, :, c0f:c0f + TB * BL],
                            in_=of[:, :, j, :])
                        nc.sync.dma_start(
                            out=out_d[1, j, :, c0b:c0b + TB * BL],
                            in_=ob[:, :, j, :])

        # ---------------- phases ----------------
        with nc.named_scope("G0"):
            wk0 = load_wk(0)
            g0 = precompute_G(
                0, wk0,
                lambda d, k, n: x0t[:, k, n * 512:(n + 1) * 512])

        with nc.named_scope("L0"):
            wr0 = load_wr(0)
            x1t = persist.tile([128, 2, NH, T, BL], BF16, tag="hfull")
            recurrence(0, wr0, g0, x1t, is_last=False)

        with nc.named_scope("G1"):
            wk1 = load_wk(1)

            def rhs1(d, k, n):
                kk = k  # contraction index over 2H = (dir, j)
                dd, jj = kk // NH, kk % NH
                flat = x1t[:, dd, jj, :, :].rearrange("p t b -> p (t b)")
                return flat[:, n * 512:(n + 1) * 512]

            g1 = precompute_G(1, wk1, rhs1)

        with nc.named_scope("L1"):
            wr1 = load_wr(1)
            h1 = persist.tile([128, 2, NH, T, BL], BF16, tag="hfull")
            recurrence(1, wr1, g1, h1, is_last=True)

    _split_excess_waits(nc)
    return nc


# ---------------------------------------------------------------------------
# Host-side input preparation
# ---------------------------------------------------------------------------
def _prep_weights(Wk, Wr, b):
    """Permute gate blocks [i,f,g,o] -> [i,f,o,2g]; return device arrays."""
    def perm(w):
        i, f, g, o = (w[..., 0:H], w[..., H:2 * H],
                      w[..., 2 * H:3 * H], w[..., 3 * H:4 * H])
        return np.concatenate([i, f, o, 2.0 * g], axis=-1)

    Wkp = perm(np.asarray(Wk, np.float32))
    Wrp = perm(np.asarray(Wr, np.float32))
    bp = perm(np.asarray(b, np.float32))
    nk = Wkp.shape[0] // 128
    wk_dev = np.ascontiguousarray(Wkp.reshape(nk, 128, 4 * H)).astype(BF16_NP)
    wr_dev = np.ascontiguousarray(Wrp.reshape(NH, 128, 4 * H)).astype(BF16_NP)
    bias_dev = np.ascontiguousarray(bp.reshape(NM, 128).T).astype(np.float32)
    return wk_dev, wr_dev, bias_dev


def make_in_maps(inputs, T=T_FULL):
    x = np.asarray(inputs["inputs"], np.float32)   # [B, T, D]
    weights = {}
    for l in range(2):
        for di, dn in enumerate(("fw", "bw")):
            wk, wr, bias = _prep_weights(inputs[f"Wk{l}_{dn}"],
                                         inputs[f"Wr{l}_{dn}"],
                                         inputs[f"b{l}_{dn}"])
            weights[f"wk{l}{di}"] = wk
            weights[f"wr{l}{di}"] = wr
            weights[f"bias{l}{di}"] = bias

    in_maps = []
    for c in range(NCORES):
        xc = x[c * BL:(c + 1) * BL]                        # [BL, T, D]
        xt = np.ascontiguousarray(xc.transpose(2, 1, 0))   # [D, T, BL]
        xt = xt.reshape(D // 128, 128, T * BL).astype(BF16_NP)
        m = {"xT": xt}
        m.update(weights)
        in_maps.append(m)
    return in_maps


_PROGRAM_CACHE = {}


def _get_program(T=T_FULL):
    if T not in _PROGRAM_CACHE:
        _PROGRAM_CACHE[T] = build_program(T=T)
    return _PROGRAM_CACHE[T]


def run(inputs, T=T_FULL, **kw):
    nc = _get_program(T)
    in_maps = make_in_maps(inputs, T=T)
    res = run_bass_kernel_spmd(nc, in_maps, core_ids=list(range(NCORES)), **kw)
    outs = []
    for r in res.results:
        o = r["out"].reshape(2, NH, 128, T, BL)       # [d, j, p, t, b]
        o = o.transpose(4, 3, 0, 1, 2)                # [b, t, d, j, p]
        outs.append(np.ascontiguousarray(o.reshape(BL, T, 2 * H)))
    out = np.concatenate(outs, axis=0)
    return out, res


def kernel(**inputs):
    out, _ = run(inputs)
    return out


if __name__ == "__main__":
    import time

    t0 = time.time()
    nc = _get_program()
    print(f"build took {time.time() - t0:.1f}s")


# revision 10
# speedup vs baseline: 1.1826x; 1.0703x over previous
"""Trainium2 Bass kernel for a 2-layer bidirectional LSTM encoder.

Problem: inputs [64, 512, 256] -> 2 stacked Bidirectional(LSTM(384)) layers
-> output [64, 512, 768] (Keras gate order i,f,g,o; sigmoid/tanh).

Strategy (8 NeuronCores, data-parallel over batch, 8 batch rows per core):
  * Everything on-chip is feature-major ("transposed"): features on the 128
    SBUF partitions, (time, batch) along the free dim.  This makes the gate
    elementwise work use all 128 vector/scalar lanes.
  * The input projections G = X @ Wk + b for all timesteps are precomputed
    with large weight-stationary matmuls and staged in DRAM (bf16).
  * The sequential recurrence then only does z_t = G_t + Wr^T h_{t-1} as 36
    small weight-stationary matmuls (12 output chunks x 3 contraction chunks)
    per direction per step, with fw/bw interleaved so the gate latency of one
    direction hides under the other direction's PE burst.
  * Host pre-permutes gates to [i, f, o, 2*g] so tanh(g) = 2*sigmoid(2g)-1
    turns ALL gate activations into a single Sigmoid instruction per step.
"""

import os
import sys

for _p in ("/opt/trn_rl_repo", "/root/.axon_site/_ro/trn_rl_repo"):
    if os.path.isdir(_p) and _p not in sys.path:
        sys.path.insert(0, _p)

import ml_dtypes
import numpy as np

import concourse.bass as bass
import concourse.mybir as mybir
import concourse.tile as tile
from concourse.bass_utils import run_bass_kernel_spmd


# ---------------------------------------------------------------------------
# Workaround: walrus CoreV3 rejects the Tile tail Drain when it carries more
# than one sem wait ("Too many sync wait commands").  Redistribute the waits
# onto single-wait SP nops.
# ---------------------------------------------------------------------------
def _apply_tile_drain_fix():
    from concourse.vector_clock import ScopedClock

    if getattr(tile.TileContext, "_drain_fix_applied", False):
        return

    def _drain_and_barrier(self, tick_clock, wait_clock):
        nc = self.nc
        drain_inst = nc.sync.drain()
        wait_clock.add_sem_waits(
            drain_inst.ins, ScopedClock({None: tick_clock.global_clock})
        )
        si = drain_inst.ins.sync_info
        if si is not None and si.on_wait:
            waits = list(si.on_wait)
            ups = list(si.on_update) if si.on_update else []
            drain_inst.ins.sync_info = mybir.SyncInfo(on_wait=[], on_update=ups)
            for w in waits:
                n = nc.sync.nop()
                n.ins.sync_info = mybir.SyncInfo(on_wait=[w], on_update=[])

        nc.all_engine_barrier()
        assert self.sems is not None
        popped = nc._tile_sem_poison_stack.pop()
        assert popped is self._sem_poison
        nc.clear_and_free_semaphores(list(self.sems.allocated().values()))
        nc.all_engine_barrier()

    tile.TileContext._drain_and_barrier = _drain_and_barrier
    tile.TileContext._drain_fix_applied = True


_apply_tile_drain_fix()


def _split_excess_waits(nc, maxw=1):
    """walrus CoreV2/V3 codegen rejects instructions carrying more than one
    sem wait ("Too many sync wait commands").  Move excess waits onto NoOps
    inserted immediately before the instruction on the same engine."""
    k = 0
    for fn in nc.m.functions:
        for bb in fn.blocks:
            insts = list(bb.instructions)
            out = []
            changed = False
            for inst in insts:
                si = getattr(inst, "sync_info", None)
                if si is not None and si.on_wait and len(si.on_wait) > maxw:
                    waits = list(si.on_wait)
                    ups = list(si.on_update) if si.on_update else []
                    for w in waits[maxw:]:
                        n = mybir.InstNoOp(name=f"xwait_{k}")
                        k += 1
                        n.engine = inst.engine
                        n.sync_info = mybir.SyncInfo(on_wait=[w], on_update=[])
                        out.append(n)
                    inst.sync_info = mybir.SyncInfo(on_wait=waits[:maxw],
                                                    on_update=ups)
                    changed = True
                out.append(inst)
            if changed:
                bb.instructions = out


# ---------------------------------------------------------------------------
# Problem constants
# ---------------------------------------------------------------------------
B, T_FULL, D, H = 64, 512, 256, 384
NCORES = 8
BL = B // NCORES          # 8 batch rows per core
NH = H // 128             # 3 recurrent contraction chunks
NM = 4 * H // 128         # 12 output (gate-feature) chunks
F32 = mybir.dt.float32
BF16 = mybir.dt.bfloat16
AF = mybir.ActivationFunctionType
ALU = mybir.AluOpType
BF16_NP = ml_dtypes.bfloat16


def build_program(T=T_FULL, TB=32):
    """Build the single-core Bass/Tile program (same NEFF runs SPMD on 8 cores)."""
    assert T % TB == 0
    NCH = (T * BL) // 512        # 512-wide column chunks of the (t, b) axis
    NKS = {0: D // 128, 1: 2 * H // 128}   # Wk contraction chunks per layer

    nc = bass.Bass("TRN2", target_bir_lowering=False, debug=False)

    # ---------------- DRAM I/O ----------------
    xT = nc.dram_tensor("xT", [D // 128, 128, T * BL], BF16, kind="ExternalInput")
    # feature-major bf16 output: out[d, j, p, t*BL + b]; host casts + transposes
    out_d = nc.dram_tensor("out", [2, NH, 128, T * BL], BF16, kind="ExternalOutput")

    wk_d, wr_d, bias_d = {}, {}, {}
    for l in range(2):
        for d in range(2):
            nk = NKS[l]
            wk_d[l, d] = nc.dram_tensor(f"wk{l}{d}", [nk, 128, 4 * H], BF16,
                                        kind="ExternalInput")
            wr_d[l, d] = nc.dram_tensor(f"wr{l}{d}", [NH, 128, 4 * H], BF16,
                                        kind="ExternalInput")
            bias_d[l, d] = nc.dram_tensor(f"bias{l}{d}", [128, NM], F32,
                                          kind="ExternalInput")

    with tile.TileContext(nc) as tc, \
         tc.tile_pool(name="persist", bufs=1) as persist, \
         tc.tile_pool(name="wkp", bufs=2) as wkp, \
         tc.tile_pool(name="wrp", bufs=2) as wrp, \
         tc.tile_pool(name="gblk", bufs=2) as gblk, \
         tc.tile_pool(name="gstage", bufs=2) as gstage, \
         tc.tile_pool(name="step", bufs=3) as stepp, \
         tc.tile_pool(name="small", bufs=4) as small, \
         tc.tile_pool(name="cells", bufs=2) as cells, \
         tc.tile_pool(name="zpsum", bufs=1, space="PSUM") as zpsum, \
         tc.tile_pool(name="ppsum", bufs=4, space="PSUM") as ppsum, \
         tc.tile_pool(name="gdram", bufs=1, space="DRAM") as gdram:

        # ---------------- constants / persistent tiles ----------------
        zero_h = persist.tile([128, BL], BF16, tag="zeroh")
        nc.vector.memset(zero_h, 0.0)

        bias_sb = {}
        for l in range(2):
            for d in range(2):
                bias_sb[l, d] = persist.tile([128, NM], F32, tag=f"bias{l}{d}", name=f"bias_sb{l}{d}")
                nc.sync.dma_start(out=bias_sb[l, d][:], in_=bias_d[l, d][:, :])

        # layer-0 input, feature-major, bf16 (host pre-transposed)
        x0t = persist.tile([128, D // 128, T * BL], BF16, tag="x0t")
        for k in range(D // 128):
            nc.sync.dma_start(out=x0t[:, k, :], in_=xT[k, :, :])

        # ---------------- helpers ----------------
        def load_wk(l):
            tiles = {}
            for d in range(2):
                nk = NKS[l]
                w = wkp.tile([128, NKS[1], 4 * H], BF16, tag="wk", name=f"wk_sb{l}{d}")
                for k in range(nk):
                    nc.sync.dma_start(out=w[:, k, :], in_=wk_d[l, d][k, :, :])
                tiles[d] = w
            return tiles

        def load_wr(l):
            tiles = {}
            for d in range(2):
                w = wrp.tile([128, NH, 4 * H], BF16, tag="wr", name=f"wr_sb{l}{d}")
                for k in range(NH):
                    nc.sync.dma_start(out=w[:, k, :], in_=wr_d[l, d][k, :, :])
                tiles[d] = w
            return tiles

        def precompute_G(l, wk_sb, rhs_fn):
            """G[d] = (X @ Wk'[d] + b'[d])^T staged to DRAM as [NM, 128, T*BL] bf16.

            rhs_fn(d, k, n) -> AP [128, 512] bf16: columns n*512..(n+1)*512 of
            the feature-major layer input, contraction chunk k.
            """
            nk = NKS[l]
            gd = {}
            for d in range(2):
                gd[d] = gdram.tile([NM, 128, T * BL], BF16, tag=f"g{l}{d}", name=f"gdram{l}{d}")
                for m in range(NM):
                    stage = gstage.tile([128, min(NCH, 4) * 512], BF16, tag="gs")
                    for ng in range((NCH + 3) // 4):
                        nlo = ng * 4
                        nhi = min(nlo + 4, NCH)
                        pss = []
                        for n in range(nlo, nhi):
                            ps = ppsum.tile([128, 512], F32, tag="pp")
                            pss.append(ps)
                            for k in range(nk):
                                nc.tensor.matmul(
                                    ps[:],
                                    wk_sb[d][:, k, m * 128:(m + 1) * 128],
                                    rhs_fn(d, k, n),
                                    start=(k == 0), stop=(k == nk - 1),
                                )
                        if ng > 0:
                            stage = gstage.tile([128, min(NCH, 4) * 512], BF16,
                                                tag="gs")
                        for i, n in enumerate(range(nlo, nhi)):
                            nc.vector.tensor_scalar_add(
                                out=stage[:, i * 512:(i + 1) * 512],
                                in0=pss[i][:],
                                scalar1=bias_sb[l, d][:, m:m + 1],
                            )
                        nc.sync.dma_start(
                            out=gd[d][m, :, nlo * 512:nhi * 512],
                            in_=stage[:, :(nhi - nlo) * 512],
                        )
            return gd

        def recurrence(l, wr_sb, g_d, hout):
            """Run T bidirectional LSTM steps for layer l.

            hout: {d: [128, NH, T, BL] bf16 tile}; h_t written feature-major.
            Per-direction h tiles keep the two chains independent so one
            direction's PE burst overlaps the other's gate chain.
            Gate chunk order is [g(0:3), i(3:6), f(6:9), o(9:12)]; the (g, i)
            half uses its own PSUM tile so its z+G add / sigmoid can start
            while the (f, o) half is still doing matmuls.
            """
            cprev = {}
            for d in range(2):
                cprev[d] = cells.tile([128, NH, BL], F32, tag=f"c{d}",
                                      name=f"cinit{d}")
                nc.vector.memset(cprev[d], 0.0)

            for blk in range(T // TB):
                gf = gblk.tile([128, NM, TB * BL], BF16, tag="gf")
                gb = gblk.tile([128, NM, TB * BL], BF16, tag="gb")
                c0 = blk * TB * BL
                nc.sync.dma_start(
                    out=gf[:],
                    in_=g_d[0][:, :, c0:c0 + TB * BL].rearrange("c p n -> p c n"))
                rb0 = T * BL - c0 - TB * BL
                nc.sync.dma_start(
                    out=gb[:],
                    in_=g_d[1][:, :, rb0:rb0 + TB * BL].rearrange("c p n -> p c n"))

                for s_ in range(TB):
                    s = blk * TB + s_
                    for d in range(2):
                        t_d = s if d == 0 else T - 1 - s
                        tprev = t_d - 1 if d == 0 else t_d + 1
                        gsl = (gf[:, :, s_ * BL:(s_ + 1) * BL] if d == 0 else
                               gb[:, :, (TB - 1 - s_) * BL:(TB - s_) * BL])

                        zpa = zpsum.tile([128, 6, BL], F32, tag=f"zpa{d}",
                                         name=f"zpa{d}_{s}")
                        zpb = zpsum.tile([128, 6, BL], F32, tag=f"zpb{d}",
                                         name=f"zpb{d}_{s}")

                        def mms(zp, clo):
                            for c in range(clo, clo + 6):
                                for k in range(NH):
                                    rhs = (zero_h[:, :] if s == 0
                                           else hout[d][:, k, tprev, :])
                                    nc.tensor.matmul(
                                        zp[:, c - clo, :],
                                        wr_sb[d][:, k, c * 128:(c + 1) * 128],
                                        rhs,
                                        start=(k == 0), stop=(k == NH - 1),
                                    )

                        # (g, i) half: matmuls, then its chain starts while
                        # the (f, o) half is still on the PE
                        mms(zpa, 0)
                        a1g = stepp.tile([128, 6, BL], F32, tag=f"a1g{d}",
                                         name=f"a1g{d}_{s}")
                        nc.vector.tensor_tensor(a1g[:], zpa[:], gsl[:, 0:6, :],
                                                ALU.add)
                        nc.scalar.activation(a1g[:], a1g[:], AF.Sigmoid)
                        gp = small.tile([128, NH, BL], F32, tag=f"gp{d}",
                                        name=f"gp{d}_{s}")
                        nc.vector.tensor_scalar(
                            out=gp[:], in0=a1g[:, 0:3, :],
                            scalar1=2.0, scalar2=1.0,
                            op0=ALU.mult, op1=ALU.subtract)
                        t1 = small.tile([128, NH, BL], F32, tag=f"t1{d}",
                                        name=f"t1{d}_{s}")
                        nc.vector.tensor_tensor(t1[:], a1g[:, 3:6, :], gp[:],
                                                ALU.mult)

                        # (f, o) half
                        mms(zpb, 6)
                        a1f = stepp.tile([128, 6, BL], F32, tag=f"a1f{d}",
                                         name=f"a1f{d}_{s}")
                        nc.vector.tensor_tensor(a1f[:], zpb[:], gsl[:, 6:12, :],
                                                ALU.add)
                        nc.scalar.activation(a1f[:], a1f[:], AF.Sigmoid)

                        t2 = small.tile([128, NH, BL], F32, tag=f"t2{d}",
                                        name=f"t2{d}_{s}")
                        nc.vector.tensor_tensor(t2[:], a1f[:, 0:3, :],
                                                cprev[d][:], ALU.mult)
                        cn = cells.tile([128, NH, BL], F32, tag=f"c{d}",
                                        name=f"c{d}_{s}")
                        nc.vector.tensor_tensor(cn[:], t1[:], t2[:], ALU.add)

                        th = small.tile([128, NH, BL], F32, tag=f"th{d}",
                                        name=f"th{d}_{s}")
                        nc.scalar.activation(th[:], cn[:], AF.Tanh)

                        # h = o * tanh(c) -> bf16, feeds next step's matmuls
                        nc.vector.tensor_tensor(hout[d][:, :, t_d, :],
                                                a1f[:, 3:6, :], th[:], ALU.mult)
                        cprev[d] = cn

        # ---------------- phases ----------------
        with nc.named_scope("G0"):
            wk0 = load_wk(0)
            g0 = precompute_G(
                0, wk0,
                lambda d, k, n: x0t[:, k, n * 512:(n + 1) * 512])

        with nc.named_scope("L0"):
            wr0 = load_wr(0)
            x1t = {}
            for d in range(2):
                x1t[d] = persist.tile([128, NH, T, BL], BF16, tag=f"hfull{d}",
                                      name=f"x1t{d}")
            recurrence(0, wr0, g0, x1t)

        with nc.named_scope("G1"):
            wk1 = load_wk(1)

            def rhs1(d, k, n):
                dd, jj = k // NH, k % NH
                flat = x1t[dd][:, jj, :, :].rearrange("p t b -> p (t b)")
                return flat[:, n * 512:(n + 1) * 512]

            g1 = precompute_G(1, wk1, rhs1)

        with nc.named_scope("L1"):
            wr1 = load_wr(1)
            h1 = {}
            for d in range(2):
                h1[d] = persist.tile([128, NH, T, BL], BF16, tag=f"hfull{d}",
                                     name=f"h1_{d}")
            recurrence(1, wr1, g1, h1)
            for d in range(2):
                for j in range(NH):
                    nc.sync.dma_start(
                        out=out_d[d, j, :, :],
                        in_=h1[d][:, j, :, :].rearrange("p t b -> p (t b)"))

    _split_excess_waits(nc)
    return nc


# ---------------------------------------------------------------------------
# Host-side input preparation
# ---------------------------------------------------------------------------
def _prep_weights(Wk, Wr, b):
    """Permute gate blocks [i,f,g,o] -> [i,f,o,2g]; return device arrays."""
    def perm(w):
        i, f, g, o = (w[..., 0:H], w[..., H:2 * H],
                      w[..., 2 * H:3 * H], w[..., 3 * H:4 * H])
        # chunk order [2g, i, f, o]: g-chunks 0-2, i 3-5, f 6-8, o 9-11
        return np.concatenate([2.0 * g, i, f, o], axis=-1)

    Wkp = perm(np.asarray(Wk, np.float32))
    Wrp = perm(np.asarray(Wr, np.float32))
    bp = perm(np.asarray(b, np.float32))
    nk = Wkp.shape[0] // 128
    wk_dev = np.ascontiguousarray(Wkp.reshape(nk, 128, 4 * H)).astype(BF16_NP)
    wr_dev = np.ascontiguousarray(Wrp.reshape(NH, 128, 4 * H)).astype(BF16_NP)
    bias_dev = np.ascontiguousarray(bp.reshape(NM, 128).T).astype(np.float32)
    return wk_dev, wr_dev, bias_dev


def make_in_maps(inputs, T=T_FULL):
    x = np.asarray(inputs["inputs"], np.float32)   # [B, T, D]
    weights = {}
    for l in range(2):
        for di, dn in enumerate(("fw", "bw")):
            wk, wr, bias = _prep_weights(inputs[f"Wk{l}_{dn}"],
                                         inputs[f"Wr{l}_{dn}"],
                                         inputs[f"b{l}_{dn}"])
            weights[f"wk{l}{di}"] = wk
            weights[f"wr{l}{di}"] = wr
            weights[f"bias{l}{di}"] = bias

    in_maps = []
    for c in range(NCORES):
        xc = x[c * BL:(c + 1) * BL]                        # [BL, T, D]
        xt = np.ascontiguousarray(xc.transpose(2, 1, 0))   # [D, T, BL]
        xt = xt.reshape(D // 128, 128, T * BL).astype(BF16_NP)
        m = {"xT": xt}
        m.update(weights)
        in_maps.append(m)
    return in_maps


_PROGRAM_CACHE = {}


def _get_program(T=T_FULL):
    if T not in _PROGRAM_CACHE:
        _PROGRAM_CACHE[T] = build_program(T=T)
    return _PROGRAM_CACHE[T]


def run(inputs, T=T_FULL, **kw):
    nc = _get_program(T)
    in_maps = make_in_maps(inputs, T=T)
    res = run_bass_kernel_spmd(nc, in_maps, core_ids=list(range(NCORES)), **kw)
    outs = []
    for r in res.results:
        o = r["out"].astype(np.float32).reshape(2, NH, 128, T, BL)  # [d,j,p,t,b]
        o = o.transpose(4, 3, 0, 1, 2)                # [b, t, d, j, p]
        outs.append(np.ascontiguousarray(o.reshape(BL, T, 2 * H)))
    out = np.concatenate(outs, axis=0)
    return out, res


def kernel(**inputs):
    out, _ = run(inputs)
    return out


if __name__ == "__main__":
    import time

    t0 = time.time()
    nc = _get_program()
    print(f"build took {time.time() - t0:.1f}s")


# revision 11
# speedup vs baseline: 1.4785x; 1.2503x over previous
"""Trainium2 Bass kernel for a 2-layer bidirectional LSTM encoder.

Problem: inputs [64, 512, 256] -> 2 stacked Bidirectional(LSTM(384)) layers
-> output [64, 512, 768] (Keras gate order i,f,g,o; sigmoid/tanh).

Strategy (8 NeuronCores, data-parallel over batch, 8 batch rows per core):
  * Everything on-chip is feature-major ("transposed"): features on the 128
    SBUF partitions, (time, batch) along the free dim.  This makes the gate
    elementwise work use all 128 vector/scalar lanes.
  * The input projections G = X @ Wk + b for all timesteps are precomputed
    with large weight-stationary matmuls and staged in DRAM (bf16).
  * The sequential recurrence then only does z_t = G_t + Wr^T h_{t-1} as 36
    small weight-stationary matmuls (12 output chunks x 3 contraction chunks)
    per direction per step, with fw/bw interleaved so the gate latency of one
    direction hides under the other direction's PE burst.
  * Host pre-permutes gates to [i, f, o, 2*g] so tanh(g) = 2*sigmoid(2g)-1
    turns ALL gate activations into a single Sigmoid instruction per step.
"""

import os
import sys

for _p in ("/opt/trn_rl_repo", "/root/.axon_site/_ro/trn_rl_repo"):
    if os.path.isdir(_p) and _p not in sys.path:
        sys.path.insert(0, _p)

import ml_dtypes
import numpy as np

import concourse.bass as bass
import concourse.mybir as mybir
import concourse.tile as tile
from concourse.bass_utils import run_bass_kernel_spmd


# ---------------------------------------------------------------------------
# Workaround: walrus CoreV3 rejects the Tile tail Drain when it carries more
# than one sem wait ("Too many sync wait commands").  Redistribute the waits
# onto single-wait SP nops.
# ---------------------------------------------------------------------------
def _apply_tile_drain_fix():
    from concourse.vector_clock import ScopedClock

    if getattr(tile.TileContext, "_drain_fix_applied", False):
        return

    def _drain_and_barrier(self, tick_clock, wait_clock):
        nc = self.nc
        drain_inst = nc.sync.drain()
        wait_clock.add_sem_waits(
            drain_inst.ins, ScopedClock({None: tick_clock.global_clock})
        )
        si = drain_inst.ins.sync_info
        if si is not None and si.on_wait:
            waits = list(si.on_wait)
            ups = list(si.on_update) if si.on_update else []
            drain_inst.ins.sync_info = mybir.SyncInfo(on_wait=[], on_update=ups)
            for w in waits:
                n = nc.sync.nop()
                n.ins.sync_info = mybir.SyncInfo(on_wait=[w], on_update=[])

        nc.all_engine_barrier()
        assert self.sems is not None
        popped = nc._tile_sem_poison_stack.pop()
        assert popped is self._sem_poison
        nc.clear_and_free_semaphores(list(self.sems.allocated().values()))
        nc.all_engine_barrier()

    tile.TileContext._drain_and_barrier = _drain_and_barrier
    tile.TileContext._drain_fix_applied = True


_apply_tile_drain_fix()


def _split_excess_waits(nc, maxw=1):
    """walrus CoreV2/V3 codegen rejects instructions carrying more than one
    sem wait ("Too many sync wait commands").  Move excess waits onto NoOps
    inserted immediately before the instruction on the same engine."""
    k = 0
    for fn in nc.m.functions:
        for bb in fn.blocks:
            insts = list(bb.instructions)
            out = []
            changed = False
            for inst in insts:
                si = getattr(inst, "sync_info", None)
                if si is not None and si.on_wait and len(si.on_wait) > maxw:
                    waits = list(si.on_wait)
                    ups = list(si.on_update) if si.on_update else []
                    for w in waits[maxw:]:
                        n = mybir.InstNoOp(name=f"xwait_{k}")
                        k += 1
                        n.engine = inst.engine
                        n.sync_info = mybir.SyncInfo(on_wait=[w], on_update=[])
                        out.append(n)
                    inst.sync_info = mybir.SyncInfo(on_wait=waits[:maxw],
                                                    on_update=ups)
                    changed = True
                out.append(inst)
            if changed:
                bb.instructions = out


# ---------------------------------------------------------------------------
# Problem constants
# ---------------------------------------------------------------------------
B, T_FULL, D, H = 64, 512, 256, 384
NCORES = 8
BL = B // NCORES          # 8 batch rows per core
NH = H // 128             # 3 recurrent contraction chunks
NM = 4 * H // 128         # 12 output (gate-feature) chunks
F32 = mybir.dt.float32
BF16 = mybir.dt.bfloat16
AF = mybir.ActivationFunctionType
ALU = mybir.AluOpType
BF16_NP = ml_dtypes.bfloat16


def build_program(T=T_FULL, TB=32):
    """Build the single-core Bass/Tile program (same NEFF runs SPMD on 8 cores)."""
    assert T % TB == 0
    NCH = (T * BL) // 512        # 512-wide column chunks of the (t, b) axis
    NKS = {0: D // 128, 1: 2 * H // 128}   # Wk contraction chunks per layer

    nc = bass.Bass("TRN2", target_bir_lowering=False, debug=False)

    # ---------------- DRAM I/O ----------------
    xT = nc.dram_tensor("xT", [D // 128, 128, T * BL], BF16, kind="ExternalInput")
    # feature-major bf16 output: out[d, j, p, t*BL + b]; host casts + transposes
    out_d = nc.dram_tensor("out", [2, NH, 128, T * BL], BF16, kind="ExternalOutput")

    ident_d = nc.dram_tensor("ident", [128, 128], BF16, kind="ExternalInput")
    wk_d, wr_d, bias_d = {}, {}, {}
    for l in range(2):
        for d in range(2):
            nk = NKS[l]
            wk_d[l, d] = nc.dram_tensor(f"wk{l}{d}", [nk, 128, 4 * H], BF16,
                                        kind="ExternalInput")
            wr_d[l, d] = nc.dram_tensor(f"wr{l}{d}", [NH, 128, 4 * H], BF16,
                                        kind="ExternalInput")
            bias_d[l, d] = nc.dram_tensor(f"bias{l}{d}", [128, NM], F32,
                                          kind="ExternalInput")

    with tile.TileContext(nc) as tc, \
         tc.tile_pool(name="persist", bufs=1) as persist, \
         tc.tile_pool(name="wkp", bufs=2) as wkp, \
         tc.tile_pool(name="wrp", bufs=2) as wrp, \
         tc.tile_pool(name="gblk", bufs=2) as gblk, \
         tc.tile_pool(name="gstage", bufs=2) as gstage, \
         tc.tile_pool(name="step", bufs=3) as stepp, \
         tc.tile_pool(name="small", bufs=4) as small, \
         tc.tile_pool(name="cells", bufs=2) as cells, \
         tc.tile_pool(name="zpsum", bufs=1, space="PSUM") as zpsum, \
         tc.tile_pool(name="ppsum", bufs=4, space="PSUM") as ppsum, \
         tc.tile_pool(name="gdram", bufs=1, space="DRAM") as gdram:

        # ---------------- constants / persistent tiles ----------------
        zero_h = persist.tile([128, BL], BF16, tag="zeroh")
        nc.vector.memset(zero_h, 0.0)
        ident = persist.tile([128, 128], BF16, tag="ident")
        nc.sync.dma_start(out=ident[:], in_=ident_d[:, :])

        bias_sb = {}
        for l in range(2):
            for d in range(2):
                bias_sb[l, d] = persist.tile([128, NM], F32, tag=f"bias{l}{d}", name=f"bias_sb{l}{d}")
                nc.sync.dma_start(out=bias_sb[l, d][:], in_=bias_d[l, d][:, :])

        # layer-0 input, feature-major, bf16 (host pre-transposed)
        x0t = persist.tile([128, D // 128, T * BL], BF16, tag="x0t")
        for k in range(D // 128):
            nc.sync.dma_start(out=x0t[:, k, :], in_=xT[k, :, :])

        # ---------------- helpers ----------------
        def load_wk(l):
            tiles = {}
            for d in range(2):
                nk = NKS[l]
                w = wkp.tile([128, NKS[1], 4 * H], BF16, tag="wk", name=f"wk_sb{l}{d}")
                for k in range(nk):
                    nc.sync.dma_start(out=w[:, k, :], in_=wk_d[l, d][k, :, :])
                tiles[d] = w
            return tiles

        def load_wr(l):
            tiles = {}
            for d in range(2):
                w = wrp.tile([128, NH, 4 * H], BF16, tag="wr", name=f"wr_sb{l}{d}")
                for k in range(NH):
                    nc.sync.dma_start(out=w[:, k, :], in_=wr_d[l, d][k, :, :])
                tiles[d] = w
            return tiles

        def precompute_G(l, wk_sb, rhs_fn):
            """G[d] = (X @ Wk'[d] + b'[d])^T staged to DRAM as [NM, 128, T*BL] bf16.

            rhs_fn(d, k, n) -> AP [128, 512] bf16: columns n*512..(n+1)*512 of
            the feature-major layer input, contraction chunk k.
            """
            nk = NKS[l]
            gd = {}
            for d in range(2):
                gd[d] = gdram.tile([NM, 128, T * BL], BF16, tag=f"g{l}{d}", name=f"gdram{l}{d}")
                for m in range(NM):
                    stage = gstage.tile([128, min(NCH, 4) * 512], BF16, tag="gs")
                    for ng in range((NCH + 3) // 4):
                        nlo = ng * 4
                        nhi = min(nlo + 4, NCH)
                        pss = []
                        for n in range(nlo, nhi):
                            ps = ppsum.tile([128, 512], F32, tag="pp")
                            pss.append(ps)
                            for k in range(nk):
                                nc.tensor.matmul(
                                    ps[:],
                                    wk_sb[d][:, k, m * 128:(m + 1) * 128],
                                    rhs_fn(d, k, n),
                                    start=(k == 0), stop=(k == nk - 1),
                                )
                        if ng > 0:
                            stage = gstage.tile([128, min(NCH, 4) * 512], BF16,
                                                tag="gs")
                        for i, n in enumerate(range(nlo, nhi)):
                            nc.vector.tensor_scalar_add(
                                out=stage[:, i * 512:(i + 1) * 512],
                                in0=pss[i][:],
                                scalar1=bias_sb[l, d][:, m:m + 1],
                            )
                        nc.sync.dma_start(
                            out=gd[d][m, :, nlo * 512:nhi * 512],
                            in_=stage[:, :(nhi - nlo) * 512],
                        )
            return gd

        def recurrence(l, wr_sb, g_d, hout):
            """Run T bidirectional LSTM steps for layer l.

            hout: {d: [128, NH, T, BL] bf16 tile}; h_t written feature-major.
            Per-direction h tiles keep the two chains independent so one
            direction's PE burst overlaps the other's gate chain.
            Gate chunk order is [g(0:3), i(3:6), f(6:9), o(9:12)]; the (g, i)
            half uses its own PSUM tile so its z+G add / sigmoid can start
            while the (f, o) half is still doing matmuls.
            """
            cprev = {}
            for d in range(2):
                cprev[d] = cells.tile([128, NH, BL], F32, tag=f"c{d}",
                                      name=f"cinit{d}")
                nc.vector.memset(cprev[d], 0.0)

            for blk in range(T // TB):
                gf = gblk.tile([128, NM, TB * BL], BF16, tag="gf")
                gb = gblk.tile([128, NM, TB * BL], BF16, tag="gb")
                c0 = blk * TB * BL
                nc.sync.dma_start(
                    out=gf[:],
                    in_=g_d[0][:, :, c0:c0 + TB * BL].rearrange("c p n -> p c n"))
                rb0 = T * BL - c0 - TB * BL
                nc.sync.dma_start(
                    out=gb[:],
                    in_=g_d[1][:, :, rb0:rb0 + TB * BL].rearrange("c p n -> p c n"))

                for s_ in range(TB):
                    s = blk * TB + s_
                    for d in range(2):
                        t_d = s if d == 0 else T - 1 - s
                        tprev = t_d - 1 if d == 0 else t_d + 1
                        gsl = (gf[:, :, s_ * BL:(s_ + 1) * BL] if d == 0 else
                               gb[:, :, (TB - 1 - s_) * BL:(TB - s_) * BL])

                        zpa = zpsum.tile([128, 6, BL], F32, tag=f"zpa{d}",
                                         name=f"zpa{d}_{s}")
                        zpb = zpsum.tile([128, 6, BL], F32, tag=f"zpb{d}",
                                         name=f"zpb{d}_{s}")

                        def mms(zp, clo):
                            # G lands in PSUM via an identity matmul, then the
                            # recurrent contributions accumulate on top.
                            nc.tensor.matmul(
                                zp[:, :, :], ident[:],
                                gsl[:, clo:clo + 6, :],
                                start=True, stop=False, skip_group_check=True)
                            for c in range(clo, clo + 6):
                                for k in range(NH):
                                    rhs = (zero_h[:, :] if s == 0
                                           else hout[d][:, k, tprev, :])
                                    nc.tensor.matmul(
                                        zp[:, c - clo, :],
                                        wr_sb[d][:, k, c * 128:(c + 1) * 128],
                                        rhs,
                                        start=False, stop=(k == NH - 1),
                                        skip_group_check=True,
                                    )

                        # (g, i) half: matmuls, then its chain starts while
                        # the (f, o) half is still on the PE
                        mms(zpa, 0)
                        a1g = stepp.tile([128, 6, BL], F32, tag=f"a1g{d}",
                                         name=f"a1g{d}_{s}")
                        nc.scalar.activation(a1g[:], zpa[:], AF.Sigmoid)
                        gp = small.tile([128, NH, BL], F32, tag=f"gp{d}",
                                        name=f"gp{d}_{s}")
                        nc.vector.tensor_scalar(
                            out=gp[:], in0=a1g[:, 0:3, :],
                            scalar1=2.0, scalar2=1.0,
                            op0=ALU.mult, op1=ALU.subtract)
                        t1 = small.tile([128, NH, BL], F32, tag=f"t1{d}",
                                        name=f"t1{d}_{s}")
                        nc.vector.tensor_tensor(t1[:], a1g[:, 3:6, :], gp[:],
                                                ALU.mult)

                        # (f, o) half
                        mms(zpb, 6)
                        a1f = stepp.tile([128, 6, BL], F32, tag=f"a1f{d}",
                                         name=f"a1f{d}_{s}")
                        nc.scalar.activation(a1f[:], zpb[:], AF.Sigmoid)

                        t2 = small.tile([128, NH, BL], F32, tag=f"t2{d}",
                                        name=f"t2{d}_{s}")
                        nc.vector.tensor_tensor(t2[:], a1f[:, 0:3, :],
                                                cprev[d][:], ALU.mult)
                        cn = cells.tile([128, NH, BL], F32, tag=f"c{d}",
                                        name=f"c{d}_{s}")
                        nc.vector.tensor_tensor(cn[:], t1[:], t2[:], ALU.add)

                        th = small.tile([128, NH, BL], F32, tag=f"th{d}",
                                        name=f"th{d}_{s}")
                        nc.scalar.activation(th[:], cn[:], AF.Tanh)

                        # h = o * tanh(c) -> bf16, feeds next step's matmuls
                        nc.vector.tensor_tensor(hout[d][:, :, t_d, :],
                                                a1f[:, 3:6, :], th[:], ALU.mult)
                        cprev[d] = cn

        # ---------------- phases ----------------
        with nc.named_scope("G0"):
            wk0 = load_wk(0)
            g0 = precompute_G(
                0, wk0,
                lambda d, k, n: x0t[:, k, n * 512:(n + 1) * 512])

        with nc.named_scope("L0"):
            wr0 = load_wr(0)
            x1t = {}
            for d in range(2):
                x1t[d] = persist.tile([128, NH, T, BL], BF16, tag=f"hfull{d}",
                                      name=f"x1t{d}")
            recurrence(0, wr0, g0, x1t)

        with nc.named_scope("G1"):
            wk1 = load_wk(1)

            def rhs1(d, k, n):
                dd, jj = k // NH, k % NH
                flat = x1t[dd][:, jj, :, :].rearrange("p t b -> p (t b)")
                return flat[:, n * 512:(n + 1) * 512]

            g1 = precompute_G(1, wk1, rhs1)

        with nc.named_scope("L1"):
            wr1 = load_wr(1)
            h1 = {}
            for d in range(2):
                h1[d] = persist.tile([128, NH, T, BL], BF16, tag=f"hfull{d}",
                                     name=f"h1_{d}")
            recurrence(1, wr1, g1, h1)
            for d in range(2):
                for j in range(NH):
                    nc.sync.dma_start(
                        out=out_d[d, j, :, :],
                        in_=h1[d][:, j, :, :].rearrange("p t b -> p (t b)"))

    _split_excess_waits(nc)
    return nc


# ---------------------------------------------------------------------------
# Host-side input preparation
# ---------------------------------------------------------------------------
def _prep_weights(Wk, Wr, b):
    """Permute gate blocks [i,f,g,o] -> [i,f,o,2g]; return device arrays."""
    def perm(w):
        i, f, g, o = (w[..., 0:H], w[..., H:2 * H],
                      w[..., 2 * H:3 * H], w[..., 3 * H:4 * H])
        # chunk order [2g, i, f, o]: g-chunks 0-2, i 3-5, f 6-8, o 9-11
        return np.concatenate([2.0 * g, i, f, o], axis=-1)

    Wkp = perm(np.asarray(Wk, np.float32))
    Wrp = perm(np.asarray(Wr, np.float32))
    bp = perm(np.asarray(b, np.float32))
    nk = Wkp.shape[0] // 128
    wk_dev = np.ascontiguousarray(Wkp.reshape(nk, 128, 4 * H)).astype(BF16_NP)
    wr_dev = np.ascontiguousarray(Wrp.reshape(NH, 128, 4 * H)).astype(BF16_NP)
    bias_dev = np.ascontiguousarray(bp.reshape(NM, 128).T).astype(np.float32)
    return wk_dev, wr_dev, bias_dev


def make_in_maps(inputs, T=T_FULL):
    x = np.asarray(inputs["inputs"], np.float32)   # [B, T, D]
    weights = {}
    for l in range(2):
        for di, dn in enumerate(("fw", "bw")):
            wk, wr, bias = _prep_weights(inputs[f"Wk{l}_{dn}"],
                                         inputs[f"Wr{l}_{dn}"],
                                         inputs[f"b{l}_{dn}"])
            weights[f"wk{l}{di}"] = wk
            weights[f"wr{l}{di}"] = wr
            weights[f"bias{l}{di}"] = bias

    in_maps = []
    for c in range(NCORES):
        xc = x[c * BL:(c + 1) * BL]                        # [BL, T, D]
        xt = np.ascontiguousarray(xc.transpose(2, 1, 0))   # [D, T, BL]
        xt = xt.reshape(D // 128, 128, T * BL).astype(BF16_NP)
        m = {"xT": xt, "ident": np.eye(128, dtype=BF16_NP)}
        m.update(weights)
        in_maps.append(m)
    return in_maps


_PROGRAM_CACHE = {}


def _get_program(T=T_FULL):
    if T not in _PROGRAM_CACHE:
        _PROGRAM_CACHE[T] = build_program(T=T)
    return _PROGRAM_CACHE[T]


def run(inputs, T=T_FULL, **kw):
    nc = _get_program(T)
    in_maps = make_in_maps(inputs, T=T)
    res = run_bass_kernel_spmd(nc, in_maps, core_ids=list(range(NCORES)), **kw)
    outs = []
    for r in res.results:
        o = r["out"].astype(np.float32).reshape(2, NH, 128, T, BL)  # [d,j,p,t,b]
        o = o.transpose(4, 3, 0, 1, 2)                # [b, t, d, j, p]
        outs.append(np.ascontiguousarray(o.reshape(BL, T, 2 * H)))
    out = np.concatenate(outs, axis=0)
    return out, res


def kernel(**inputs):
    out, _ = run(inputs)
    return out


if __name__ == "__main__":
    import time

    t0 = time.time()
    nc = _get_program()
    print(f"build took {time.time() - t0:.1f}s")


# revision 12
# speedup vs baseline: 1.5555x; 1.0520x over previous
"""Trainium2 Bass kernel for a 2-layer bidirectional LSTM encoder.

Problem: inputs [64, 512, 256] -> 2 stacked Bidirectional(LSTM(384)) layers
-> output [64, 512, 768] (Keras gate order i,f,g,o; sigmoid/tanh).

Strategy (8 NeuronCores, data-parallel over batch, 8 batch rows per core):
  * Everything on-chip is feature-major ("transposed"): features on the 128
    SBUF partitions, (time, batch) along the free dim.  This makes the gate
    elementwise work use all 128 vector/scalar lanes.
  * The input projections G = X @ Wk + b for all timesteps are precomputed
    with large weight-stationary matmuls and staged in DRAM (bf16).
  * The sequential recurrence then only does z_t = G_t + Wr^T h_{t-1} as 36
    small weight-stationary matmuls (12 output chunks x 3 contraction chunks)
    per direction per step, with fw/bw interleaved so the gate latency of one
    direction hides under the other direction's PE burst.
  * Host pre-permutes gates to [i, f, o, 2*g] so tanh(g) = 2*sigmoid(2g)-1
    turns ALL gate activations into a single Sigmoid instruction per step.
"""

import os
import sys

for _p in ("/opt/trn_rl_repo", "/root/.axon_site/_ro/trn_rl_repo"):
    if os.path.isdir(_p) and _p not in sys.path:
        sys.path.insert(0, _p)

import ml_dtypes
import numpy as np

import concourse.bass as bass
import concourse.mybir as mybir
import concourse.tile as tile
from concourse.bass_utils import run_bass_kernel_spmd


# ---------------------------------------------------------------------------
# Workaround: walrus CoreV3 rejects the Tile tail Drain when it carries more
# than one sem wait ("Too many sync wait commands").  Redistribute the waits
# onto single-wait SP nops.
# ---------------------------------------------------------------------------
def _apply_tile_drain_fix():
    from concourse.vector_clock import ScopedClock

    if getattr(tile.TileContext, "_drain_fix_applied", False):
        return

    def _drain_and_barrier(self, tick_clock, wait_clock):
        nc = self.nc
        drain_inst = nc.sync.drain()
        wait_clock.add_sem_waits(
            drain_inst.ins, ScopedClock({None: tick_clock.global_clock})
        )
        si = drain_inst.ins.sync_info
        if si is not None and si.on_wait:
            waits = list(si.on_wait)
            ups = list(si.on_update) if si.on_update else []
            drain_inst.ins.sync_info = mybir.SyncInfo(on_wait=[], on_update=ups)
            for w in waits:
                n = nc.sync.nop()
                n.ins.sync_info = mybir.SyncInfo(on_wait=[w], on_update=[])

        nc.all_engine_barrier()
        assert self.sems is not None
        popped = nc._tile_sem_poison_stack.pop()
        assert popped is self._sem_poison
        nc.clear_and_free_semaphores(list(self.sems.allocated().values()))
        nc.all_engine_barrier()

    tile.TileContext._drain_and_barrier = _drain_and_barrier
    tile.TileContext._drain_fix_applied = True


_apply_tile_drain_fix()


def _split_excess_waits(nc, maxw=1):
    """walrus CoreV2/V3 codegen rejects instructions carrying more than one
    sem wait ("Too many sync wait commands").  Move excess waits onto NoOps
    inserted immediately before the instruction on the same engine."""
    k = 0
    for fn in nc.m.functions:
        for bb in fn.blocks:
            insts = list(bb.instructions)
            out = []
            changed = False
            for inst in insts:
                si = getattr(inst, "sync_info", None)
                if si is not None and si.on_wait and len(si.on_wait) > maxw:
                    waits = list(si.on_wait)
                    ups = list(si.on_update) if si.on_update else []
                    for w in waits[maxw:]:
                        n = mybir.InstNoOp(name=f"xwait_{k}")
                        k += 1
                        n.engine = inst.engine
                        n.sync_info = mybir.SyncInfo(on_wait=[w], on_update=[])
                        out.append(n)
                    inst.sync_info = mybir.SyncInfo(on_wait=waits[:maxw],
                                                    on_update=ups)
                    changed = True
                out.append(inst)
            if changed:
                bb.instructions = out


# ---------------------------------------------------------------------------
# Problem constants
# ---------------------------------------------------------------------------
B, T_FULL, D, H = 64, 512, 256, 384
NCORES = 8
BL = B // NCORES          # 8 batch rows per core
NH = H // 128             # 3 recurrent contraction chunks
NM = 4 * H // 128         # 12 output (gate-feature) chunks
F32 = mybir.dt.float32
BF16 = mybir.dt.bfloat16
AF = mybir.ActivationFunctionType
ALU = mybir.AluOpType
BF16_NP = ml_dtypes.bfloat16


def build_program(T=T_FULL, TB=32):
    """Build the single-core Bass/Tile program (same NEFF runs SPMD on 8 cores)."""
    assert T % TB == 0
    NCH = (T * BL) // 512        # 512-wide column chunks of the (t, b) axis
    NKS = {0: D // 128, 1: 2 * H // 128}   # Wk contraction chunks per layer

    nc = bass.Bass("TRN2", target_bir_lowering=False, debug=False)

    # ---------------- DRAM I/O ----------------
    xT = nc.dram_tensor("xT", [D // 128, 128, T * BL], BF16, kind="ExternalInput")
    # feature-major bf16 output: out[d, j, p, t*BL + b]; host casts + transposes
    out_d = nc.dram_tensor("out", [2, NH, 128, T * BL], BF16, kind="ExternalOutput")

    ident_d = nc.dram_tensor("ident", [128, 128], BF16, kind="ExternalInput")
    wk_d, wr_d, bias_d = {}, {}, {}
    for l in range(2):
        for d in range(2):
            nk = NKS[l]
            wk_d[l, d] = nc.dram_tensor(f"wk{l}{d}", [nk, 128, 4 * H], BF16,
                                        kind="ExternalInput")
            wr_d[l, d] = nc.dram_tensor(f"wr{l}{d}", [NH, 128, 4 * H], BF16,
                                        kind="ExternalInput")
            bias_d[l, d] = nc.dram_tensor(f"bias{l}{d}", [128, NM], F32,
                                          kind="ExternalInput")

    with tile.TileContext(nc) as tc, \
         tc.tile_pool(name="persist", bufs=1) as persist, \
         tc.tile_pool(name="wkp", bufs=2) as wkp, \
         tc.tile_pool(name="wrp", bufs=2) as wrp, \
         tc.tile_pool(name="gblk", bufs=2) as gblk, \
         tc.tile_pool(name="gstage", bufs=2) as gstage, \
         tc.tile_pool(name="step", bufs=3) as stepp, \
         tc.tile_pool(name="small", bufs=4) as small, \
         tc.tile_pool(name="cells", bufs=2) as cells, \
         tc.tile_pool(name="zpsum", bufs=1, space="PSUM") as zpsum, \
         tc.tile_pool(name="ppsum", bufs=2, space="PSUM") as ppsum, \
         tc.tile_pool(name="gdram", bufs=1, space="DRAM") as gdram:

        # ---------------- constants / persistent tiles ----------------
        zero_h = persist.tile([128, BL], BF16, tag="zeroh")
        nc.vector.memset(zero_h, 0.0)
        ident = persist.tile([128, 128], BF16, tag="ident")
        nc.sync.dma_start(out=ident[:], in_=ident_d[:, :])

        bias_sb = {}
        for l in range(2):
            for d in range(2):
                bias_sb[l, d] = persist.tile([128, NM], F32, tag=f"bias{l}{d}", name=f"bias_sb{l}{d}")
                nc.sync.dma_start(out=bias_sb[l, d][:], in_=bias_d[l, d][:, :])

        # layer-0 input, feature-major, bf16 (host pre-transposed)
        x0t = persist.tile([128, D // 128, T * BL], BF16, tag="x0t")
        for k in range(D // 128):
            nc.sync.dma_start(out=x0t[:, k, :], in_=xT[k, :, :])

        # ---------------- helpers ----------------
        def load_wk(l):
            tiles = {}
            for d in range(2):
                nk = NKS[l]
                w = wkp.tile([128, NKS[1], 4 * H], BF16, tag="wk", name=f"wk_sb{l}{d}")
                for k in range(nk):
                    nc.sync.dma_start(out=w[:, k, :], in_=wk_d[l, d][k, :, :])
                tiles[d] = w
            return tiles

        def load_wr(l):
            tiles = {}
            for d in range(2):
                w = wrp.tile([128, NH, 4 * H], BF16, tag="wr", name=f"wr_sb{l}{d}")
                for k in range(NH):
                    nc.sync.dma_start(out=w[:, k, :], in_=wr_d[l, d][k, :, :])
                tiles[d] = w
            return tiles

        def precompute_G(l, wk_sb, rhs_fn):
            """G[d] = (X @ Wk'[d] + b'[d])^T staged to DRAM as [NM, 128, T*BL] bf16.

            rhs_fn(d, k, n) -> AP [128, 512] bf16: columns n*512..(n+1)*512 of
            the feature-major layer input, contraction chunk k.
            """
            nk = NKS[l]
            gd = {}
            for d in range(2):
                gd[d] = gdram.tile([NM, 128, T * BL], BF16, tag=f"g{l}{d}", name=f"gdram{l}{d}")
                for m in range(NM):
                    stage = gstage.tile([128, min(NCH, 2) * 512], BF16, tag="gs")
                    for ng in range((NCH + 1) // 2):
                        nlo = ng * 2
                        nhi = min(nlo + 2, NCH)
                        pss = []
                        for n in range(nlo, nhi):
                            ps = ppsum.tile([128, 512], F32, tag="pp")
                            pss.append(ps)
                            for k in range(nk):
                                nc.tensor.matmul(
                                    ps[:],
                                    wk_sb[d][:, k, m * 128:(m + 1) * 128],
                                    rhs_fn(d, k, n),
                                    start=(k == 0), stop=(k == nk - 1),
                                )
                        if ng > 0:
                            stage = gstage.tile([128, min(NCH, 2) * 512], BF16,
                                                tag="gs")
                        for i, n in enumerate(range(nlo, nhi)):
                            nc.vector.tensor_scalar_add(
                                out=stage[:, i * 512:(i + 1) * 512],
                                in0=pss[i][:],
                                scalar1=bias_sb[l, d][:, m:m + 1],
                            )
                        nc.sync.dma_start(
                            out=gd[d][m, :, nlo * 512:nhi * 512],
                            in_=stage[:, :(nhi - nlo) * 512],
                        )
            return gd

        def recurrence(l, wr_sb, g_d, hout):
            """Run T bidirectional LSTM steps for layer l.

            hout: {d: [128, NH, T, BL] bf16 tile}; h_t written feature-major.
            Per-direction h tiles keep the two chains independent so one
            direction's PE burst overlaps the other's gate chain.
            Gate chunk order is [g(0:3), i(3:6), f(6:9), o(9:12)]; the (g, i)
            half uses its own PSUM tile so its z+G add / sigmoid can start
            while the (f, o) half is still doing matmuls.
            """
            cprev = {}
            for d in range(2):
                cprev[d] = cells.tile([128, NH, BL], F32, tag=f"c{d}",
                                      name=f"cinit{d}")
                nc.vector.memset(cprev[d], 0.0)

            for blk in range(T // TB):
                gf = gblk.tile([128, NM, TB * BL], BF16, tag="gf")
                gb = gblk.tile([128, NM, TB * BL], BF16, tag="gb")
                c0 = blk * TB * BL
                nc.sync.dma_start(
                    out=gf[:],
                    in_=g_d[0][:, :, c0:c0 + TB * BL].rearrange("c p n -> p c n"))
                rb0 = T * BL - c0 - TB * BL
                nc.sync.dma_start(
                    out=gb[:],
                    in_=g_d[1][:, :, rb0:rb0 + TB * BL].rearrange("c p n -> p c n"))

                for s_ in range(TB):
                    s = blk * TB + s_
                    for d in range(2):
                        t_d = s if d == 0 else T - 1 - s
                        tprev = t_d - 1 if d == 0 else t_d + 1
                        gsl = (gf[:, :, s_ * BL:(s_ + 1) * BL] if d == 0 else
                               gb[:, :, (TB - 1 - s_) * BL:(TB - s_) * BL])

                        zpa = zpsum.tile([128, 6, BL], F32, tag=f"zpa{d}",
                                         name=f"zpa{d}_{s}")
                        zpf = zpsum.tile([128, 3, BL], F32, tag=f"zpf{d}",
                                         name=f"zpf{d}_{s}")
                        zpo = zpsum.tile([128, 3, BL], F32, tag=f"zpo{d}",
                                         name=f"zpo{d}_{s}")

                        def mms(zp, clo, nch):
                            # G lands in PSUM via an identity matmul, then the
                            # recurrent contributions accumulate on top.
                            nc.tensor.matmul(
                                zp[:, :, :], ident[:],
                                gsl[:, clo:clo + nch, :],
                                start=True, stop=False, skip_group_check=True)
                            for c in range(clo, clo + nch):
                                for k in range(NH):
                                    rhs = (zero_h[:, :] if s == 0
                                           else hout[d][:, k, tprev, :])
                                    nc.tensor.matmul(
                                        zp[:, c - clo, :],
                                        wr_sb[d][:, k, c * 128:(c + 1) * 128],
                                        rhs,
                                        start=False, stop=(k == NH - 1),
                                        skip_group_check=True,
                                    )

                        # (g, i): its sigmoid/gate chain starts while (f, o)
                        # matmuls still run on the PE
                        mms(zpa, 0, 6)
                        a1g = stepp.tile([128, 6, BL], F32, tag=f"a1g{d}",
                                         name=f"a1g{d}_{s}")
                        nc.scalar.activation(a1g[:], zpa[:], AF.Sigmoid)
                        gp = small.tile([128, NH, BL], F32, tag=f"gp{d}",
                                        name=f"gp{d}_{s}")
                        nc.vector.tensor_scalar(
                            out=gp[:], in0=a1g[:, 0:3, :],
                            scalar1=2.0, scalar2=1.0,
                            op0=ALU.mult, op1=ALU.subtract)
                        t1 = small.tile([128, NH, BL], F32, tag=f"t1{d}",
                                        name=f"t1{d}_{s}")
                        nc.vector.tensor_tensor(t1[:], a1g[:, 3:6, :], gp[:],
                                                ALU.mult)

                        # f gate
                        mms(zpf, 6, 3)
                        a1ff = stepp.tile([128, 3, BL], F32, tag=f"a1ff{d}",
                                          name=f"a1ff{d}_{s}")
                        nc.scalar.activation(a1ff[:], zpf[:], AF.Sigmoid)
                        t2 = small.tile([128, NH, BL], F32, tag=f"t2{d}",
                                        name=f"t2{d}_{s}")
                        nc.vector.tensor_tensor(t2[:], a1ff[:], cprev[d][:],
                                                ALU.mult)
                        cn = cells.tile([128, NH, BL], F32, tag=f"c{d}",
                                        name=f"c{d}_{s}")
                        nc.vector.tensor_tensor(cn[:], t1[:], t2[:], ALU.add)
                        th = small.tile([128, NH, BL], F32, tag=f"th{d}",
                                        name=f"th{d}_{s}")
                        nc.scalar.activation(th[:], cn[:], AF.Tanh)

                        # o gate (only needed at the very end)
                        mms(zpo, 9, 3)
                        a1o = stepp.tile([128, 3, BL], F32, tag=f"a1o{d}",
                                         name=f"a1o{d}_{s}")
                        nc.scalar.activation(a1o[:], zpo[:], AF.Sigmoid)

                        # h = o * tanh(c) -> bf16, feeds next step's matmuls
                        nc.vector.tensor_tensor(hout[d][:, :, t_d, :],
                                                a1o[:], th[:], ALU.mult)
                        cprev[d] = cn

        # ---------------- phases ----------------
        with nc.named_scope("G0"):
            wk0 = load_wk(0)
            g0 = precompute_G(
                0, wk0,
                lambda d, k, n: x0t[:, k, n * 512:(n + 1) * 512])

        with nc.named_scope("L0"):
            wr0 = load_wr(0)
            x1t = {}
            for d in range(2):
                x1t[d] = persist.tile([128, NH, T, BL], BF16, tag=f"hfull{d}",
                                      name=f"x1t{d}")
            recurrence(0, wr0, g0, x1t)

        with nc.named_scope("G1"):
            wk1 = load_wk(1)

            def rhs1(d, k, n):
                dd, jj = k // NH, k % NH
                flat = x1t[dd][:, jj, :, :].rearrange("p t b -> p (t b)")
                return flat[:, n * 512:(n + 1) * 512]

            g1 = precompute_G(1, wk1, rhs1)

        with nc.named_scope("L1"):
            wr1 = load_wr(1)
            h1 = {}
            for d in range(2):
                h1[d] = persist.tile([128, NH, T, BL], BF16, tag=f"hfull{d}",
                                     name=f"h1_{d}")
            recurrence(1, wr1, g1, h1)
            for d in range(2):
                for j in range(NH):
                    nc.sync.dma_start(
                        out=out_d[d, j, :, :],
                        in_=h1[d][:, j, :, :].rearrange("p t b -> p (t b)"))

    _split_excess_waits(nc)
    return nc


# ---------------------------------------------------------------------------
# Host-side input preparation
# ---------------------------------------------------------------------------
def _prep_weights(Wk, Wr, b):
    """Permute gate blocks [i,f,g,o] -> [i,f,o,2g]; return device arrays."""
    def perm(w):
        i, f, g, o = (w[..., 0:H], w[..., H:2 * H],
                      w[..., 2 * H:3 * H], w[..., 3 * H:4 * H])
        # chunk order [2g, i, f, o]: g-chunks 0-2, i 3-5, f 6-8, o 9-11
        return np.concatenate([2.0 * g, i, f, o], axis=-1)

    Wkp = perm(np.asarray(Wk, np.float32))
    Wrp = perm(np.asarray(Wr, np.float32))
    bp = perm(np.asarray(b, np.float32))
    nk = Wkp.shape[0] // 128
    wk_dev = np.ascontiguousarray(Wkp.reshape(nk, 128, 4 * H)).astype(BF16_NP)
    wr_dev = np.ascontiguousarray(Wrp.reshape(NH, 128, 4 * H)).astype(BF16_NP)
    bias_dev = np.ascontiguousarray(bp.reshape(NM, 128).T).astype(np.float32)
    return wk_dev, wr_dev, bias_dev


def make_in_maps(inputs, T=T_FULL):
    x = np.asarray(inputs["inputs"], np.float32)   # [B, T, D]
    weights = {}
    for l in range(2):
        for di, dn in enumerate(("fw", "bw")):
            wk, wr, bias = _prep_weights(inputs[f"Wk{l}_{dn}"],
                                         inputs[f"Wr{l}_{dn}"],
                                         inputs[f"b{l}_{dn}"])
            weights[f"wk{l}{di}"] = wk
            weights[f"wr{l}{di}"] = wr
            weights[f"bias{l}{di}"] = bias

    in_maps = []
    for c in range(NCORES):
        xc = x[c * BL:(c + 1) * BL]                        # [BL, T, D]
        xt = np.ascontiguousarray(xc.transpose(2, 1, 0))   # [D, T, BL]
        xt = xt.reshape(D // 128, 128, T * BL).astype(BF16_NP)
        m = {"xT": xt, "ident": np.eye(128, dtype=BF16_NP)}
        m.update(weights)
        in_maps.append(m)
    return in_maps


_PROGRAM_CACHE = {}


def _get_program(T=T_FULL):
    if T not in _PROGRAM_CACHE:
        _PROGRAM_CACHE[T] = build_program(T=T)
    return _PROGRAM_CACHE[T]


def run(inputs, T=T_FULL, **kw):
    nc = _get_program(T)
    in_maps = make_in_maps(inputs, T=T)
    res = run_bass_kernel_spmd(nc, in_maps, core_ids=list(range(NCORES)), **kw)
    outs = []
    for r in res.results:
        o = r["out"].astype(np.float32).reshape(2, NH, 128, T, BL)  # [d,j,p,t,b]
        o = o.transpose(4, 3, 0, 1, 2)                # [b, t, d, j, p]
        outs.append(np.ascontiguousarray(o.reshape(BL, T, 2 * H)))
    out = np.concatenate(outs, axis=0)
    return out, res


def kernel(**inputs):
    out, _ = run(inputs)
    return out


if __name__ == "__main__":
    import time

    t0 = time.time()
    nc = _get_program()
    print(f"build took {time.time() - t0:.1f}s")


# revision 13
# speedup vs baseline: 1.5935x; 1.0245x over previous
"""Trainium2 Bass kernel for a 2-layer bidirectional LSTM encoder.

Problem: inputs [64, 512, 256] -> 2 stacked Bidirectional(LSTM(384)) layers
-> output [64, 512, 768] (Keras gate order i,f,g,o; sigmoid/tanh).

Strategy (8 NeuronCores, data-parallel over batch, 8 batch rows per core):
  * Everything on-chip is feature-major ("transposed"): features on the 128
    SBUF partitions, (time, batch) along the free dim.  This makes the gate
    elementwise work use all 128 vector/scalar lanes.
  * The input projections G = X @ Wk + b for all timesteps are precomputed
    with large weight-stationary matmuls and staged in DRAM (bf16).
  * The sequential recurrence then only does z_t = G_t + Wr^T h_{t-1} as 36
    small weight-stationary matmuls (12 output chunks x 3 contraction chunks)
    per direction per step, with fw/bw interleaved so the gate latency of one
    direction hides under the other direction's PE burst.
  * Host pre-permutes gates to [i, f, o, 2*g] so tanh(g) = 2*sigmoid(2g)-1
    turns ALL gate activations into a single Sigmoid instruction per step.
"""

import os
import sys

for _p in ("/opt/trn_rl_repo", "/root/.axon_site/_ro/trn_rl_repo"):
    if os.path.isdir(_p) and _p not in sys.path:
        sys.path.insert(0, _p)

import ml_dtypes
import numpy as np

import concourse.bass as bass
import concourse.mybir as mybir
import concourse.tile as tile
from concourse.bass_utils import run_bass_kernel_spmd


# ---------------------------------------------------------------------------
# Workaround: walrus CoreV3 rejects the Tile tail Drain when it carries more
# than one sem wait ("Too many sync wait commands").  Redistribute the waits
# onto single-wait SP nops.
# ---------------------------------------------------------------------------
def _apply_tile_drain_fix():
    from concourse.vector_clock import ScopedClock

    if getattr(tile.TileContext, "_drain_fix_applied", False):
        return

    def _drain_and_barrier(self, tick_clock, wait_clock):
        nc = self.nc
        drain_inst = nc.sync.drain()
        wait_clock.add_sem_waits(
            drain_inst.ins, ScopedClock({None: tick_clock.global_clock})
        )
        si = drain_inst.ins.sync_info
        if si is not None and si.on_wait:
            waits = list(si.on_wait)
            ups = list(si.on_update) if si.on_update else []
            drain_inst.ins.sync_info = mybir.SyncInfo(on_wait=[], on_update=ups)
            for w in waits:
                n = nc.sync.nop()
                n.ins.sync_info = mybir.SyncInfo(on_wait=[w], on_update=[])

        nc.all_engine_barrier()
        assert self.sems is not None
        popped = nc._tile_sem_poison_stack.pop()
        assert popped is self._sem_poison
        nc.clear_and_free_semaphores(list(self.sems.allocated().values()))
        nc.all_engine_barrier()

    tile.TileContext._drain_and_barrier = _drain_and_barrier
    tile.TileContext._drain_fix_applied = True


_apply_tile_drain_fix()


def _split_excess_waits(nc, maxw=1):
    """walrus CoreV2/V3 codegen rejects instructions carrying more than one
    sem wait ("Too many sync wait commands").  Move excess waits onto NoOps
    inserted immediately before the instruction on the same engine."""
    k = 0
    for fn in nc.m.functions:
        for bb in fn.blocks:
            insts = list(bb.instructions)
            out = []
            changed = False
            for inst in insts:
                si = getattr(inst, "sync_info", None)
                if si is not None and si.on_wait and len(si.on_wait) > maxw:
                    waits = list(si.on_wait)
                    ups = list(si.on_update) if si.on_update else []
                    for w in waits[maxw:]:
                        n = mybir.InstNoOp(name=f"xwait_{k}")
                        k += 1
                        n.engine = inst.engine
                        n.sync_info = mybir.SyncInfo(on_wait=[w], on_update=[])
                        out.append(n)
                    inst.sync_info = mybir.SyncInfo(on_wait=waits[:maxw],
                                                    on_update=ups)
                    changed = True
                out.append(inst)
            if changed:
                bb.instructions = out


# ---------------------------------------------------------------------------
# Problem constants
# ---------------------------------------------------------------------------
B, T_FULL, D, H = 64, 512, 256, 384
NCORES = 8
BL = B // NCORES          # 8 batch rows per core
NH = H // 128             # 3 recurrent contraction chunks
NM = 4 * H // 128         # 12 output (gate-feature) chunks
F32 = mybir.dt.float32
BF16 = mybir.dt.bfloat16
AF = mybir.ActivationFunctionType
ALU = mybir.AluOpType
BF16_NP = ml_dtypes.bfloat16


def build_program(T=T_FULL, TB=32):
    """Build the single-core Bass/Tile program (same NEFF runs SPMD on 8 cores)."""
    assert T % TB == 0
    NCH = (T * BL) // 512        # 512-wide column chunks of the (t, b) axis
    NKS = {0: D // 128, 1: 2 * H // 128}   # Wk contraction chunks per layer

    nc = bass.Bass("TRN2", target_bir_lowering=False, debug=False)

    # ---------------- DRAM I/O ----------------
    xT = nc.dram_tensor("xT", [D // 128, 128, T * BL], BF16, kind="ExternalInput")
    # feature-major bf16 output: out[d, j, p, t*BL + b]; host casts + transposes
    out_d = nc.dram_tensor("out", [2, NH, 128, T * BL], BF16, kind="ExternalOutput")

    ident_d = nc.dram_tensor("ident", [128, 128], BF16, kind="ExternalInput")
    wk_d, wr_d, bias_d = {}, {}, {}
    for l in range(2):
        for d in range(2):
            nk = NKS[l]
            wk_d[l, d] = nc.dram_tensor(f"wk{l}{d}", [nk, 128, 4 * H], BF16,
                                        kind="ExternalInput")
            wr_d[l, d] = nc.dram_tensor(f"wr{l}{d}", [NH, 128, 4 * H], BF16,
                                        kind="ExternalInput")
            bias_d[l, d] = nc.dram_tensor(f"bias{l}{d}", [128, NM], F32,
                                          kind="ExternalInput")

    with tile.TileContext(nc) as tc, \
         tc.tile_pool(name="persist", bufs=1) as persist, \
         tc.tile_pool(name="wkp", bufs=2) as wkp, \
         tc.tile_pool(name="wrp", bufs=2) as wrp, \
         tc.tile_pool(name="gblk", bufs=3) as gblk, \
         tc.tile_pool(name="gstage", bufs=2) as gstage, \
         tc.tile_pool(name="step", bufs=3) as stepp, \
         tc.tile_pool(name="small", bufs=4) as small, \
         tc.tile_pool(name="cells", bufs=2) as cells, \
         tc.tile_pool(name="zpsum", bufs=1, space="PSUM") as zpsum, \
         tc.tile_pool(name="ppsum", bufs=2, space="PSUM") as ppsum, \
         tc.tile_pool(name="gdram", bufs=1, space="DRAM") as gdram:

        # ---------------- constants / persistent tiles ----------------
        zero_h = persist.tile([128, BL], BF16, tag="zeroh")
        nc.vector.memset(zero_h, 0.0)
        ident = persist.tile([128, 128], BF16, tag="ident")
        nc.sync.dma_start(out=ident[:], in_=ident_d[:, :])

        bias_sb = {}
        for l in range(2):
            for d in range(2):
                bias_sb[l, d] = persist.tile([128, NM], F32, tag=f"bias{l}{d}", name=f"bias_sb{l}{d}")
                nc.sync.dma_start(out=bias_sb[l, d][:], in_=bias_d[l, d][:, :])

        # layer-0 input, feature-major, bf16 (host pre-transposed)
        x0t = persist.tile([128, D // 128, T * BL], BF16, tag="x0t")
        for k in range(D // 128):
            nc.sync.dma_start(out=x0t[:, k, :], in_=xT[k, :, :])

        # ---------------- helpers ----------------
        def load_wk(l):
            tiles = {}
            for d in range(2):
                nk = NKS[l]
                w = wkp.tile([128, NKS[1], 4 * H], BF16, tag="wk", name=f"wk_sb{l}{d}")
                for k in range(nk):
                    nc.sync.dma_start(out=w[:, k, :], in_=wk_d[l, d][k, :, :])
                tiles[d] = w
            return tiles

        def load_wr(l):
            tiles = {}
            for d in range(2):
                w = wrp.tile([128, NH, 4 * H], BF16, tag="wr", name=f"wr_sb{l}{d}")
                for k in range(NH):
                    nc.sync.dma_start(out=w[:, k, :], in_=wr_d[l, d][k, :, :])
                tiles[d] = w
            return tiles

        def precompute_G(l, wk_sb, rhs_fn):
            """G[d] = (X @ Wk'[d] + b'[d])^T staged to DRAM as [NM, 128, T*BL] bf16.

            rhs_fn(d, k, n) -> AP [128, 512] bf16: columns n*512..(n+1)*512 of
            the feature-major layer input, contraction chunk k.
            """
            nk = NKS[l]
            gd = {}
            for d in range(2):
                gd[d] = gdram.tile([NM, 128, T * BL], BF16, tag=f"g{l}{d}",
                                   name=f"gdram{l}{d}")
            # Column-group outer, fw ascending / bw descending: the first
            # recurrence block of each direction unblocks after one group.
            ngt = (NCH + 1) // 2
            for ngi in range(ngt):
                for d in range(2):
                    ng = ngi if d == 0 else ngt - 1 - ngi
                    nlo = ng * 2
                    nhi = min(nlo + 2, NCH)
                    for m in range(NM):
                        pss = []
                        for n in range(nlo, nhi):
                            ps = ppsum.tile([128, 512], F32, tag="pp")
                            pss.append(ps)
                            for k in range(nk):
                                nc.tensor.matmul(
                                    ps[:],
                                    wk_sb[d][:, k, m * 128:(m + 1) * 128],
                                    rhs_fn(d, k, n),
                                    start=(k == 0), stop=(k == nk - 1),
                                )
                        stage = gstage.tile([128, min(NCH, 2) * 512], BF16,
                                            tag="gs")
                        for i, n in enumerate(range(nlo, nhi)):
                            nc.vector.tensor_scalar_add(
                                out=stage[:, i * 512:(i + 1) * 512],
                                in0=pss[i][:],
                                scalar1=bias_sb[l, d][:, m:m + 1],
                            )
                        nc.sync.dma_start(
                            out=gd[d][m, :, nlo * 512:nhi * 512],
                            in_=stage[:, :(nhi - nlo) * 512],
                        )
            return gd

        def recurrence(l, wr_sb, g_d, hout):
            """Run T bidirectional LSTM steps for layer l.

            hout: {d: [128, NH, T, BL] bf16 tile}; h_t written feature-major.
            Per-direction h tiles keep the two chains independent so one
            direction's PE burst overlaps the other's gate chain.
            Gate chunk order is [g(0:3), i(3:6), f(6:9), o(9:12)]; the (g, i)
            half uses its own PSUM tile so its z+G add / sigmoid can start
            while the (f, o) half is still doing matmuls.
            """
            cprev = {}
            for d in range(2):
                cprev[d] = cells.tile([128, NH, BL], F32, tag=f"c{d}",
                                      name=f"cinit{d}")
                nc.vector.memset(cprev[d], 0.0)

            for blk in range(T // TB):
                gf = gblk.tile([128, NM, TB * BL], BF16, tag="gf")
                gb = gblk.tile([128, NM, TB * BL], BF16, tag="gb")
                c0 = blk * TB * BL
                nc.sync.dma_start(
                    out=gf[:],
                    in_=g_d[0][:, :, c0:c0 + TB * BL].rearrange("c p n -> p c n"))
                rb0 = T * BL - c0 - TB * BL
                nc.sync.dma_start(
                    out=gb[:],
                    in_=g_d[1][:, :, rb0:rb0 + TB * BL].rearrange("c p n -> p c n"))

                for s_ in range(TB):
                    s = blk * TB + s_
                    for d in range(2):
                        t_d = s if d == 0 else T - 1 - s
                        tprev = t_d - 1 if d == 0 else t_d + 1
                        gsl = (gf[:, :, s_ * BL:(s_ + 1) * BL] if d == 0 else
                               gb[:, :, (TB - 1 - s_) * BL:(TB - s_) * BL])

                        zpa = zpsum.tile([128, 6, BL], F32, tag=f"zpa{d}",
                                         name=f"zpa{d}_{s}")
                        zpf = zpsum.tile([128, 3, BL], F32, tag=f"zpf{d}",
                                         name=f"zpf{d}_{s}")
                        zpo = zpsum.tile([128, 3, BL], F32, tag=f"zpo{d}",
                                         name=f"zpo{d}_{s}")

                        def mms(zp, clo, nch):
                            # G lands in PSUM via an identity matmul, then the
                            # recurrent contributions accumulate on top.
                            nc.tensor.matmul(
                                zp[:, :, :], ident[:],
                                gsl[:, clo:clo + nch, :],
                                start=True, stop=False, skip_group_check=True)
                            for c in range(clo, clo + nch):
                                for k in range(NH):
                                    rhs = (zero_h[:, :] if s == 0
                                           else hout[d][:, k, tprev, :])
                                    nc.tensor.matmul(
                                        zp[:, c - clo, :],
                                        wr_sb[d][:, k, c * 128:(c + 1) * 128],
                                        rhs,
                                        start=False, stop=(k == NH - 1),
                                        skip_group_check=True,
                                    )

                        # (g, i): its sigmoid/gate chain starts while (f, o)
                        # matmuls still run on the PE
                        mms(zpa, 0, 6)
                        a1g = stepp.tile([128, 6, BL], F32, tag=f"a1g{d}",
                                         name=f"a1g{d}_{s}")
                        nc.scalar.activation(a1g[:], zpa[:], AF.Sigmoid)
                        gp = small.tile([128, NH, BL], F32, tag=f"gp{d}",
                                        name=f"gp{d}_{s}")
                        nc.vector.tensor_scalar(
                            out=gp[:], in0=a1g[:, 0:3, :],
                            scalar1=2.0, scalar2=1.0,
                            op0=ALU.mult, op1=ALU.subtract)
                        t1 = small.tile([128, NH, BL], F32, tag=f"t1{d}",
                                        name=f"t1{d}_{s}")
                        nc.vector.tensor_tensor(t1[:], a1g[:, 3:6, :], gp[:],
                                                ALU.mult)

                        # f gate
                        mms(zpf, 6, 3)
                        a1ff = stepp.tile([128, 3, BL], F32, tag=f"a1ff{d}",
                                          name=f"a1ff{d}_{s}")
                        nc.scalar.activation(a1ff[:], zpf[:], AF.Sigmoid)
                        t2 = small.tile([128, NH, BL], F32, tag=f"t2{d}",
                                        name=f"t2{d}_{s}")
                        nc.vector.tensor_tensor(t2[:], a1ff[:], cprev[d][:],
                                                ALU.mult)
                        cn = cells.tile([128, NH, BL], F32, tag=f"c{d}",
                                        name=f"c{d}_{s}")
                        nc.vector.tensor_tensor(cn[:], t1[:], t2[:], ALU.add)
                        th = small.tile([128, NH, BL], F32, tag=f"th{d}",
                                        name=f"th{d}_{s}")
                        nc.scalar.activation(th[:], cn[:], AF.Tanh)

                        # o gate (only needed at the very end)
                        mms(zpo, 9, 3)
                        a1o = stepp.tile([128, 3, BL], F32, tag=f"a1o{d}",
                                         name=f"a1o{d}_{s}")
                        nc.scalar.activation(a1o[:], zpo[:], AF.Sigmoid)

                        # h = o * tanh(c) -> bf16, feeds next step's matmuls
                        nc.vector.tensor_tensor(hout[d][:, :, t_d, :],
                                                a1o[:], th[:], ALU.mult)
                        cprev[d] = cn

        # ---------------- phases ----------------
        with nc.named_scope("G0"):
            wk0 = load_wk(0)
            g0 = precompute_G(
                0, wk0,
                lambda d, k, n: x0t[:, k, n * 512:(n + 1) * 512])

        with nc.named_scope("L0"):
            wr0 = load_wr(0)
            x1t = {}
            for d in range(2):
                x1t[d] = persist.tile([128, NH, T, BL], BF16, tag=f"hfull{d}",
                                      name=f"x1t{d}")
            recurrence(0, wr0, g0, x1t)

        with nc.named_scope("G1"):
            wk1 = load_wk(1)

            def rhs1(d, k, n):
                dd, jj = k // NH, k % NH
                flat = x1t[dd][:, jj, :, :].rearrange("p t b -> p (t b)")
                return flat[:, n * 512:(n + 1) * 512]

            g1 = precompute_G(1, wk1, rhs1)

        with nc.named_scope("L1"):
            wr1 = load_wr(1)
            h1 = {}
            for d in range(2):
                h1[d] = persist.tile([128, NH, T, BL], BF16, tag=f"hfull{d}",
                                     name=f"h1_{d}")
            recurrence(1, wr1, g1, h1)
            for d in range(2):
                for j in range(NH):
                    nc.sync.dma_start(
                        out=out_d[d, j, :, :],
                        in_=h1[d][:, j, :, :].rearrange("p t b -> p (t b)"))

    _split_excess_waits(nc)
    return nc


# ---------------------------------------------------------------------------
# Host-side input preparation
# ---------------------------------------------------------------------------
def _prep_weights(Wk, Wr, b):
    """Permute gate blocks [i,f,g,o] -> [i,f,o,2g]; return device arrays."""
    def perm(w):
        i, f, g, o = (w[..., 0:H], w[..., H:2 * H],
                      w[..., 2 * H:3 * H], w[..., 3 * H:4 * H])
        # chunk order [2g, i, f, o]: g-chunks 0-2, i 3-5, f 6-8, o 9-11
        return np.concatenate([2.0 * g, i, f, o], axis=-1)

    Wkp = perm(np.asarray(Wk, np.float32))
    Wrp = perm(np.asarray(Wr, np.float32))
    bp = perm(np.asarray(b, np.float32))
    nk = Wkp.shape[0] // 128
    wk_dev = np.ascontiguousarray(Wkp.reshape(nk, 128, 4 * H)).astype(BF16_NP)
    wr_dev = np.ascontiguousarray(Wrp.reshape(NH, 128, 4 * H)).astype(BF16_NP)
    bias_dev = np.ascontiguousarray(bp.reshape(NM, 128).T).astype(np.float32)
    return wk_dev, wr_dev, bias_dev


def make_in_maps(inputs, T=T_FULL):
    x = np.asarray(inputs["inputs"], np.float32)   # [B, T, D]
    weights = {}
    for l in range(2):
        for di, dn in enumerate(("fw", "bw")):
            wk, wr, bias = _prep_weights(inputs[f"Wk{l}_{dn}"],
                                         inputs[f"Wr{l}_{dn}"],
                                         inputs[f"b{l}_{dn}"])
            weights[f"wk{l}{di}"] = wk
            weights[f"wr{l}{di}"] = wr
            weights[f"bias{l}{di}"] = bias

    in_maps = []
    for c in range(NCORES):
        xc = x[c * BL:(c + 1) * BL]                        # [BL, T, D]
        xt = np.ascontiguousarray(xc.transpose(2, 1, 0))   # [D, T, BL]
        xt = xt.reshape(D // 128, 128, T * BL).astype(BF16_NP)
        m = {"xT": xt, "ident": np.eye(128, dtype=BF16_NP)}
        m.update(weights)
        in_maps.append(m)
    return in_maps


_PROGRAM_CACHE = {}


def _get_program(T=T_FULL):
    if T not in _PROGRAM_CACHE:
        _PROGRAM_CACHE[T] = build_program(T=T)
    return _PROGRAM_CACHE[T]


def run(inputs, T=T_FULL, **kw):
    nc = _get_program(T)
    in_maps = make_in_maps(inputs, T=T)
    res = run_bass_kernel_spmd(nc, in_maps, core_ids=list(range(NCORES)), **kw)
    outs = []
    for r in res.results:
        o = r["out"].astype(np.float32).reshape(2, NH, 128, T, BL)  # [d,j,p,t,b]
        o = o.transpose(4, 3, 0, 1, 2)                # [b, t, d, j, p]
        outs.append(np.ascontiguousarray(o.reshape(BL, T, 2 * H)))
    out = np.concatenate(outs, axis=0)
    return out, res


def kernel(**inputs):
    out, _ = run(inputs)
    return out


if __name__ == "__main__":
    import time

    t0 = time.time()
    nc = _get_program()
    print(f"build took {time.time() - t0:.1f}s")
